# revision 1
# baseline (speedup 1.0000x reference)
"""Trainium2 Bass kernel for the MIOSTONE tree model (8-core SPMD).

Strategy
--------
The two big weight matrices are block-diagonal (tree structure:
``kron(eye(n), ones(H, K*ipc))``), so the dense 772 MB of weights carry only
~5.6 MB of real data.  Host-side we extract the diagonal blocks and shard by
subtree: core ``c`` owns depth-1 node ``c`` (64 depth-3 nodes, 8 depth-2
nodes, 1 depth-1 node).  All activations live on-chip as
[feature-on-partition, batch-on-free] so layers chain without transposes.
The only cross-core coupling (root layer + batchnorm + output projection) is
handled with an HBM AllGather of the 8 per-core [64, 32] tails, after which
every core redundantly computes the tiny root/BN/projection; core 0's output
is returned.

The gate combine ``x = g*relu(z_m) + (1-g)*x_lin`` is folded into the packed
weights: with s = g/(1-g) >= 0 we can pull the scale inside the relu, so the
per-layer combine is a single tensor add in a 1/(1-g)-scaled basis.  BN is
scale-invariant up to eps (compensated via eps' = eps/(1-g)^2) and the sign
of (1-g) (folded into gamma).  A fallback "direct" mode handles degenerate
gates with one extra scaled copy per tile.

Hardware constraints shaping the emission:
- A matmul (fused fp32 LDW+MM) can encode at most ONE sync wait, so every
  matmul may depend on at most one "processor" Tile hasn't already observed
  on PE.  Therefore: all DMAs ride the single SWDGE queue (one semaphore),
  inputs arrive in two order-chained blob DMAs, all psum drains/combines run
  on the vector engine only, and a dummy matmul after depth-3 absorbs the
  second blob's queue tick before depth-2 matmuls need it.
- Matmul psum/stationary base partitions are limited to {0, 32, 64}: depth-3
  lhsT tiles are stacked 3-high (bases 0/32/64) in 128-partition blob
  columns, depth-2 packs 2 nodes per [64, 32] psum tile.
"""

import numpy as np

import concourse.bacc as bacc
import concourse.bass as bass
import concourse.mybir as mybir
import concourse.tile as tile
from bass_rust import add_dep_helper
from concourse.bass_utils import run_bass_kernel_spmd

NCORES = 8
EPS = 1e-5
F32 = mybir.dt.float32
AF = mybir.ActivationFunctionType
ALU = mybir.AluOpType

# blob 1 (dma #1): biases/misc + xt + w3   [128, N1]
C_B3 = 0          # [128, 48]  cols m:0-15 l:16-31 lc:32-47
C_B2 = 48         # [64, 12]   m:0-3 l:4-7 lc:8-11
C_B1 = 60         # [32, 3]
C_B0 = 63         # [32, 3]
C_BN = 66         # [32, 2]    gamma', beta
C_WO = 68         # [33, 2]    [Wout.T ; bout]
C_XT = 70         # 16 tiles of [32, 32] at row base 32*(t%3), col 32*t
C_W3 = C_XT + 512  # 12 col-blocks of 128; tile (br,t) at block 6*br+t//3, row base 32*(t%3)
N1 = C_W3 + 12 * 128

# blob 2 (dma #2): w2 + w1 + w0   [128, N2]
C_W2 = 0          # [128, 1024]  (( br*8 + j)*2 + s)*32
C_W1 = 1024       # [64, 256]    (br*4 + ch)*32
C_W0 = 1280       # [128, 128]   (br*2 + k)*32
N2 = C_W0 + 128


def _extract_blocks(w, n, rows, cols):
    """Diagonal blocks of block-diag matrix w: out[i] = w[i*rows:(i+1)*rows, i*cols:(i+1)*cols]."""
    s0, s1 = w.strides
    return np.lib.stride_tricks.as_strided(
        w, (n, rows, cols), (rows * s0 + cols * s1, s0, s1)
    ).copy()


def _build_module(scaled: bool, g: float, debug: bool = False) -> bass.Bass:
    """Emit the per-core SPMD Bass module (identical program on all 8 cores)."""
    nc = bacc.Bacc(num_devices=NCORES)

    in1_d = nc.dram_tensor("in1", [128, N1], F32, kind="ExternalInput")
    in2_d = nc.dram_tensor("in2", [128, N2], F32, kind="ExternalInput")
    out_d = nc.dram_tensor("out", [32, 2], F32, kind="ExternalOutput")
    dbg_d = {}
    if debug:
        for nm, shp in [("d_u3", [128, 512]), ("d_xl3", [128, 512]),
                        ("d_u2", [64, 128]), ("d_xl2", [64, 128]),
                        ("d_u1", [32, 32]), ("d_xl1", [32, 32]),
                        ("d_ccout", [512, 32]), ("d_x1f", [128, 64]),
                        ("d_xl1f", [128, 64]), ("d_x0", [32, 32])]:
            dbg_d[nm] = nc.dram_tensor(nm, shp, F32, kind="ExternalOutput")

    eps_c = EPS / (1.0 - g) ** 2 if scaled else EPS
    sub_combine = (not scaled) and g < 0.0

    with tile.TileContext(nc) as tc:
        with (
            tc.tile_pool(name="weights", bufs=1) as wp,
            tc.tile_pool(name="acts", bufs=1) as acp,
            tc.tile_pool(name="scratch", bufs=4) as sp,
            tc.tile_pool(name="small", bufs=2) as smp,
            tc.tile_pool(name="psumL", bufs=4, space="PSUM") as pL,
            tc.tile_pool(name="psumS", bufs=4, space="PSUM") as pS,
            tc.tile_pool(name="dram", bufs=1, space="DRAM") as dp,
        ):
            in1 = wp.tile([128, N1], F32, name="in1_sb")
            dma1 = nc.gpsimd.dma_start(in1[:, :], in1_d[:, :])
            in2 = wp.tile([128, N2], F32, name="in2_sb")
            dma2 = nc.gpsimd.dma_start(in2[:, :], in2_d[:, :])
            add_dep_helper(dma2.ins, dma1.ins, False, "queue order: blob1 first")

            # slices of the input blobs
            def xt_t(t):
                rb = 32 * (t % 3)
                return in1[rb : rb + 32, C_XT + 32 * t : C_XT + 32 * (t + 1)]

            def w3_t(br, t):
                rb = 32 * (t % 3)
                cb = 6 * br + t // 3
                return in1[rb : rb + 32, C_W3 + 128 * cb : C_W3 + 128 * (cb + 1)]

            def b3c(kind, t):
                return in1[:, C_B3 + 16 * kind + t : C_B3 + 16 * kind + t + 1]

            def b2c(kind, pp):
                return in1[0:64, C_B2 + 4 * kind + pp : C_B2 + 4 * kind + pp + 1]

            def b1c(kind):
                return in1[0:32, C_B1 + kind : C_B1 + kind + 1]

            def b0c(kind):
                return in1[0:32, C_B0 + kind : C_B0 + kind + 1]

            def w2s(br, j, s):
                o = ((br * 8 + j) * 2 + s) * 32
                return in2[:, C_W2 + o : C_W2 + o + 32]

            def w1s(br, ch):
                o = (br * 4 + ch) * 32
                return in2[0:64, C_W1 + o : C_W1 + o + 32]

            def w0s(br, k):
                o = (br * 2 + k) * 32
                return in2[:, C_W0 + o : C_W0 + o + 32]

            # persistent activation buffers (feature-on-partition, batch-on-free)
            u3 = acp.tile([128, 512], F32, name="u3_sb")
            xl3 = acp.tile([128, 512], F32, name="xl3_sb")
            u2 = acp.tile([64, 4, 32], F32, name="u2_sb")
            xl2 = acp.tile([64, 4, 32], F32, name="xl2_sb")
            u1 = acp.tile([32, 32], F32, name="u1_sb")
            xl1 = acp.tile([32, 32], F32, name="xl1_sb")

            def drain_relu(dst, psum, bias_col):
                # dst = max(psum + bias, 0) on DVE
                nc.vector.tensor_scalar(
                    dst, psum, bias_col, 0.0, op0=ALU.add, op1=ALU.max
                )

            def drain_lin(dst, psum, bias_col):
                nc.vector.tensor_scalar(dst, psum, bias_col, None, op0=ALU.add)

            def combine(dst, hm_t, xl_dst, psl_t, bias_col_dir):
                """dst = (+-)hm + scaled-chain; scaled mode: chain==xl already drained."""
                if scaled:
                    nc.vector.tensor_add(dst, hm_t, xl_dst)
                else:
                    xlc = sp.tile(
                        [psl_t.shape[0], 32], F32, name="xlc", tag="xlc"
                    )
                    nc.vector.tensor_scalar(
                        xlc[:, :], psl_t, 1.0 - g, bias_col_dir,
                        op0=ALU.mult, op1=ALU.add,
                    )
                    if sub_combine:
                        nc.vector.tensor_sub(dst, xlc[:, :], hm_t)
                    else:
                        nc.vector.tensor_add(dst, hm_t, xlc[:, :])

            # ---- depth-3: 16 M-tiles of 128 features (4 nodes, K=32 true) ----
            for t in range(16):
                psm = pL.tile([128, 32], F32, name="psm", tag="psL")
                psl = pL.tile([128, 32], F32, name="psl", tag="psL")
                nc.tensor.matmul(
                    psm[:, :], lhsT=w3_t(0, t), rhs=xt_t(t), start=True, stop=True
                )
                nc.tensor.matmul(
                    psl[:, :], lhsT=w3_t(1, t), rhs=xt_t(t), start=True, stop=True
                )
                hm = sp.tile([128, 32], F32, name="hm", tag="hm")
                drain_relu(hm[:, :], psm[:, :], b3c(0, t))
                xl_dst = xl3[:, t * 32 : (t + 1) * 32]
                drain_lin(xl_dst, psl[:, :], b3c(1, t))
                combine(u3[:, t * 32 : (t + 1) * 32], hm[:, :], xl_dst, psl[:, :],
                        b3c(2, t))

            # dummy matmul: absorbs blob2's queue tick onto PE before depth-2
            psd = pS.tile([32, 2], F32, name="psd", tag="psS")
            nc.tensor.matmul(
                psd[:, :], lhsT=in2[0:32, 0:32], rhs=in2[0:32, 0:2],
                start=True, stop=True,
            )

            # ---- depth-2: 4 pairs of nodes, [64, 32] psum per pair ----
            for pp in range(4):
                ps2m = pL.tile([64, 32], F32, name="ps2m", tag="psL")
                ps2l = pL.tile([64, 32], F32, name="ps2l", tag="psL")
                for jj in range(2):
                    j = 2 * pp + jj
                    for s in range(2):
                        nc.tensor.matmul(
                            ps2m[32 * jj : 32 * (jj + 1), :],
                            lhsT=w2s(0, j, s),
                            rhs=u3[:, (2 * j + s) * 32 : (2 * j + s + 1) * 32],
                            start=(s == 0), stop=(s == 1),
                        )
                    for s in range(2):
                        nc.tensor.matmul(
                            ps2l[32 * jj : 32 * (jj + 1), :],
                            lhsT=w2s(1, j, s),
                            rhs=xl3[:, (2 * j + s) * 32 : (2 * j + s + 1) * 32],
                            start=(s == 0), stop=(s == 1),
                        )
                hm2 = sp.tile([64, 32], F32, name="hm2", tag="hm")
                drain_relu(hm2[:, :], ps2m[:, :], b2c(0, pp))
                xl_dst = xl2[:, pp, :]
                drain_lin(xl_dst, ps2l[:, :], b2c(1, pp))
                combine(u2[:, pp, :], hm2[:, :], xl_dst, ps2l[:, :], b2c(2, pp))

            # ---- depth-1: this core's single node (K=256 as 4 chunks of 64) ----
            ps1m = pS.tile([32, 32], F32, name="ps1m", tag="psS")
            ps1l = pS.tile([32, 32], F32, name="ps1l", tag="psS")
            for ch in range(4):
                nc.tensor.matmul(
                    ps1m[:, :], lhsT=w1s(0, ch), rhs=u2[:, ch, :],
                    start=(ch == 0), stop=(ch == 3),
                )
            for ch in range(4):
                nc.tensor.matmul(
                    ps1l[:, :], lhsT=w1s(1, ch), rhs=xl2[:, ch, :],
                    start=(ch == 0), stop=(ch == 3),
                )
            hm1 = smp.tile([32, 32], F32, name="hm1", tag="hm1")
            drain_relu(hm1[:, :], ps1m[:, :], b1c(0))
            drain_lin(xl1[:, :], ps1l[:, :], b1c(1))
            combine(u1[:, :], hm1[:, :], xl1[:, :], ps1l[:, :], b1c(2))

            # ---- AllGather the per-core tails: [u1 ; xl1] -> [8, 2, 32, 32] ----
            cc_in = dp.tile([64, 32], F32, name="cc_in")
            cc_out = dp.tile([512, 32], F32, name="cc_out")
            nc.gpsimd.dma_start(cc_in[0:32, :], u1[:, :])
            nc.gpsimd.dma_start(cc_in[32:64, :], xl1[:, :])
            nc.gpsimd.collective_compute(
                "AllGather", ALU.bypass,
                replica_groups=[list(range(NCORES))],
                ins=[cc_in[:, :]], outs=[cc_out[:, :]],
            )
            ccv = cc_out[:, :].rearrange("(gc st o) b -> gc st o b", gc=8, st=2)
            x1f = acp.tile([128, 2, 32], F32, name="x1f")
            xl1f = acp.tile([128, 2, 32], F32, name="xl1f")
            # order-chained so the last read (x1f k=0) covers all queue ticks
            reads = []
            for st, dst in ((1, xl1f), (0, x1f)):
                for k in (1, 0):
                    # dst flattened (p=32*gg+o, b) order == src (gg, o, b) order
                    r = nc.gpsimd.dma_start(
                        dst[:, k, :],
                        ccv[4 * k : 4 * (k + 1), st, :, :],
                    )
                    if reads:
                        add_dep_helper(r.ins, reads[-1].ins, False, "gather order")
                    reads.append(r)

            # ---- depth-0 (root), replicated on every core ----
            ps0m = pS.tile([32, 32], F32, name="ps0m", tag="psS")
            ps0l = pS.tile([32, 32], F32, name="ps0l", tag="psS")
            for k in range(2):
                nc.tensor.matmul(
                    ps0m[:, :], lhsT=w0s(0, k), rhs=x1f[:, k, :],
                    start=(k == 0), stop=(k == 1),
                )
            for k in range(2):
                nc.tensor.matmul(
                    ps0l[:, :], lhsT=w0s(1, k), rhs=xl1f[:, k, :],
                    start=(k == 0), stop=(k == 1),
                )
            hm0 = smp.tile([32, 32], F32, name="hm0", tag="hm0")
            xl0 = smp.tile([32, 32], F32, name="xl0", tag="xl0")
            x0 = smp.tile([32, 32], F32, name="x0", tag="x0")
            drain_relu(hm0[:, :], ps0m[:, :], b0c(0))
            drain_lin(xl0[:, :], ps0l[:, :], b0c(1))
            combine(x0[:, :], hm0[:, :], xl0[:, :], ps0l[:, :], b0c(2))

            # ---- batchnorm over the batch (free) axis ----
            stats = smp.tile([32, 6], F32, name="stats", tag="stats")
            mv = smp.tile([32, 2], F32, name="mv", tag="mv")
            nc.vector.bn_stats(stats[:, :], x0[:, :])
            nc.vector.bn_aggr(mv[:, :], stats[:, :])
            eps_t = wp.tile([32, 1], F32, name="eps_t")
            nc.vector.memset(eps_t[:, :], eps_c)
            sq = smp.tile([32, 1], F32, name="sq", tag="sq")
            nc.scalar.activation(
                sq[:, :], mv[:, 1:2], AF.Sqrt, bias=eps_t[:, :], scale=1.0
            )
            rstd = smp.tile([32, 1], F32, name="rstd", tag="rstd")
            nc.vector.reciprocal(rstd[:, :], sq[:, :])

            aug = acp.tile([33, 32], F32, name="aug")
            nc.vector.memset(aug[32:33, :], 1.0)
            nc.vector.tensor_scalar(
                aug[0:32, :], x0[:, :], mv[:, 0:1], rstd[:, :],
                op0=ALU.subtract, op1=ALU.mult,
            )
            nc.vector.tensor_scalar(
                aug[0:32, :], aug[0:32, :],
                in1[0:32, C_BN : C_BN + 1], in1[0:32, C_BN + 1 : C_BN + 2],
                op0=ALU.mult, op1=ALU.add,
            )
            pso = pS.tile([32, 2], F32, name="pso", tag="psS")
            nc.tensor.matmul(
                pso[:, :], lhsT=aug[:, :], rhs=in1[0:33, C_WO : C_WO + 2],
                start=True, stop=True,
            )
            outt = smp.tile([32, 2], F32, name="outt", tag="outt")
            nc.vector.tensor_copy(outt[:, :], pso[:, :])
            nc.gpsimd.dma_start(out_d[:, :], outt[:, :])
            if debug:
                for ap_src, nm in [(u3[:, :], "d_u3"), (xl3[:, :], "d_xl3"),
                                   (u2[:, :, :], "d_u2"), (xl2[:, :, :], "d_xl2"),
                                   (u1[:, :], "d_u1"), (xl1[:, :], "d_xl1"),
                                   (cc_out[:, :], "d_ccout"),
                                   (x1f[:, :, :], "d_x1f"), (xl1f[:, :, :], "d_xl1f"),
                                   (x0[:, :], "d_x0")]:
                    nc.gpsimd.dma_start(dbg_d[nm][:, :], ap_src)

    nc.finalize()
    return nc


_module_cache: dict = {}


def _get_module(scaled: bool, g: float, debug: bool = False) -> bass.Bass:
    key = (scaled, round(float(g), 12), debug)
    if key not in _module_cache:
        _module_cache[key] = _build_module(scaled, g, debug)
    return _module_cache[key]


def _pack_inputs(x, Wm3, bm3, Wl3, bl3, Wm2, bm2, Wl2, bl2, Wm1, bm1, Wl1, bl1,
                 Wm0, bm0, Wl0, bl0, gate, bn_gamma, bn_beta, Wout, bout,
                 scaled, g):
    f = np.float32
    if scaled:
        aW3 = g / (1.0 - g)  # relu-branch weight factor, d3 (raw input basis)
        aW = g               # relu-branch weight factor, d2/d1/d0 (u basis)
        ab = g / (1.0 - g)   # relu-branch bias factor, all layers
        sgn1mg = 1.0 if (1.0 - g) > 0 else -1.0
    else:
        aW3 = aW = ab = abs(g)
        sgn1mg = 1.0

    bl3m = _extract_blocks(np.asarray(Wm3, f), 128, 128, 32)  # (128, 128m, 32k)
    bl3l = _extract_blocks(np.asarray(Wl3, f), 128, 128, 32)
    bl2m = _extract_blocks(np.asarray(Wm2, f), 64, 32, 256)   # (64, 32m, 256k)
    bl2l = _extract_blocks(np.asarray(Wl2, f), 64, 32, 256)
    bl1m = _extract_blocks(np.asarray(Wm1, f), 8, 32, 256)
    bl1l = _extract_blocks(np.asarray(Wl1, f), 8, 32, 256)
    Wm0 = np.asarray(Wm0, f)
    Wl0 = np.asarray(Wl0, f)
    x = np.asarray(x, f)
    bm3 = np.asarray(bm3, f); bl3 = np.asarray(bl3, f)
    bm2 = np.asarray(bm2, f); bl2 = np.asarray(bl2, f)
    bm1 = np.asarray(bm1, f); bl1 = np.asarray(bl1, f)
    bm0 = np.asarray(bm0, f); bl0 = np.asarray(bl0, f)

    # blob 2 is identical on every core except w2/w1 (per-core nodes); w0 shared
    w0blk = np.zeros((128, 128), f)
    for k in range(2):
        w0blk[:, (0 * 2 + k) * 32 : (0 * 2 + k + 1) * 32] = (
            aW * Wm0[:, 128 * k : 128 * (k + 1)]
        ).T
        w0blk[:, (1 * 2 + k) * 32 : (1 * 2 + k + 1) * 32] = Wl0[
            :, 128 * k : 128 * (k + 1)
        ].T

    in_maps = []
    for c in range(NCORES):
        in1 = np.zeros((128, N1), f)
        in2 = np.zeros((128, N2), f)
        # biases
        for t in range(16):
            T = 16 * c + t
            in1[:, C_B3 + t] = ab * bm3[128 * T : 128 * (T + 1)]
            in1[:, C_B3 + 16 + t] = bl3[128 * T : 128 * (T + 1)]
            in1[:, C_B3 + 32 + t] = (1.0 - g) * bl3[128 * T : 128 * (T + 1)]
        for pp in range(4):
            lo = 256 * c + 64 * pp
            in1[0:64, C_B2 + pp] = ab * bm2[lo : lo + 64]
            in1[0:64, C_B2 + 4 + pp] = bl2[lo : lo + 64]
            in1[0:64, C_B2 + 8 + pp] = (1.0 - g) * bl2[lo : lo + 64]
        in1[0:32, C_B1 + 0] = ab * bm1[32 * c : 32 * (c + 1)]
        in1[0:32, C_B1 + 1] = bl1[32 * c : 32 * (c + 1)]
        in1[0:32, C_B1 + 2] = (1.0 - g) * bl1[32 * c : 32 * (c + 1)]
        in1[0:32, C_B0 + 0] = ab * bm0
        in1[0:32, C_B0 + 1] = bl0
        in1[0:32, C_B0 + 2] = (1.0 - g) * bl0
        in1[0:32, C_BN] = sgn1mg * np.asarray(bn_gamma, f)
        in1[0:32, C_BN + 1] = np.asarray(bn_beta, f)
        in1[0:32, C_WO : C_WO + 2] = np.asarray(Wout, f).T
        in1[32, C_WO : C_WO + 2] = np.asarray(bout, f)
        # xt: tile t at rows 32*(t%3), cols C_XT + 32t; [k, b] = x[b, leaf]
        xc = x[:, 512 * c : 512 * (c + 1)]
        for t in range(16):
            rb = 32 * (t % 3)
            in1[rb : rb + 32, C_XT + 32 * t : C_XT + 32 * (t + 1)] = xc[
                :, 32 * t : 32 * (t + 1)
            ].T
        # w3: lhsT tile (br, t) = scaled_block[T].T at rows 32*(t%3), block col 6*br + t//3
        for t in range(16):
            T = 16 * c + t
            rb = 32 * (t % 3)
            cbm = C_W3 + 128 * (t // 3)
            cbl = C_W3 + 128 * (6 + t // 3)
            in1[rb : rb + 32, cbm : cbm + 128] = (aW3 * bl3m[T]).T
            in1[rb : rb + 32, cbl : cbl + 128] = bl3l[T].T
        # w2: lhsT (br, j, s) = block[n2][:, 128s:128(s+1)].T
        for j in range(8):
            n2 = 8 * c + j
            for s in range(2):
                o = C_W2 + ((0 * 8 + j) * 2 + s) * 32
                in2[:, o : o + 32] = (aW * bl2m[n2][:, 128 * s : 128 * (s + 1)]).T
                o = C_W2 + ((1 * 8 + j) * 2 + s) * 32
                in2[:, o : o + 32] = bl2l[n2][:, 128 * s : 128 * (s + 1)].T
        # w1: lhsT (br, ch) = block[c][:, 64ch:64(ch+1)].T  (64 rows)
        for ch in range(4):
            o = C_W1 + (0 * 4 + ch) * 32
            in2[0:64, o : o + 32] = (aW * bl1m[c][:, 64 * ch : 64 * (ch + 1)]).T
            o = C_W1 + (1 * 4 + ch) * 32
            in2[0:64, o : o + 32] = bl1l[c][:, 64 * ch : 64 * (ch + 1)].T
        in2[:, C_W0 : C_W0 + 128] = w0blk
        in_maps.append({"in1": in1, "in2": in2})
    return in_maps


def kernel(x, Wm3, bm3, Wl3, bl3, Wm2, bm2, Wl2, bl2, Wm1, bm1, Wl1, bl1,
           Wm0, bm0, Wl0, bl0, gate, bn_gamma, bn_beta, Wout, bout,
           _trace=False, _trace_kwargs=None, _debug=False):
    g = float(np.asarray(gate))
    scaled = abs(1.0 - g) > 1e-6 and (g / (1.0 - g)) >= 0.0
    nc = _get_module(scaled, g, _debug)
    in_maps = _pack_inputs(
        x, Wm3, bm3, Wl3, bl3, Wm2, bm2, Wl2, bl2, Wm1, bm1, Wl1, bl1,
        Wm0, bm0, Wl0, bl0, gate, bn_gamma, bn_beta, Wout, bout, scaled, g,
    )
    kwargs = dict(_trace_kwargs or {})
    res = run_bass_kernel_spmd(
        nc, in_maps, core_ids=list(range(NCORES)), trace=_trace, **kwargs
    )
    out = np.asarray(res.results[0]["out"], np.float32)
    if _debug:
        return out, res
    if _trace:
        return out, res
    return out



# revision 20
# speedup vs baseline: 1.7731x; 1.7731x over previous
"""Trainium2 Bass kernel for the MIOSTONE tree model (8-core SPMD).

Strategy (v2: fully replicated, collective-free)
------------------------------------------------
The two big weight matrices are block-diagonal (tree structure:
``kron(eye(n), ones(H, K*ipc))``), so the dense 772 MB of weights carry only
~5.6 MB of real data.  Profiling the previous (subtree-sharded + AllGather)
version showed the device collective path costs ~60 us fixed (a ~50 us CC
barrier plus a ~10 us 64 KB AllGather) on top of ~15 us of compute — the
cross-core coupling (root layer + batchnorm over the batch) is unavoidable
in any sharded scheme, so the collective dominates.  This version instead
REPLICATES the whole (compact) model on every core: each core redundantly
computes the full tree from ~4.9 MB of fp16-packed weights, and core 0's
output is returned.  No cross-core traffic at all.

Numerics: weights/activations are fp16 (PSUM accumulation stays fp32) —
the relative-error budget (2e-2) is ~40x above fp16's ~5e-4 roundoff.
fp16 also halves HBM traffic and runs the PE at full (1-pass) rate with
fast weight loads, vs fp32's multi-pass matmuls.

The gate combine ``x = g*relu(z_m) + (1-g)*x_lin`` is folded into the packed
weights: with s = g/(1-g) >= 0 the scale moves inside the relu, so the
per-layer combine is a single tensor add in a 1/(1-g)-scaled basis.  BN is
scale-invariant up to eps (compensated via eps' = eps/(1-g)^2) and the sign
of (1-g) (folded into gamma).  A fallback "direct" mode handles degenerate
gates with one extra scaled copy per drain group.

Emission structure (per core, identical SPMD program):
- depth-3: 128 "bands" x 2 branches; band = [33, 128] fp16 stationary
  (4 tree nodes' 32x8 blocks on the diagonal + a ones-row carrying the
  bias), rhs = [33, 32] transposed-x tile.  16 bands accumulate side by
  side into one [128, 512] PSUM bank, so drains are 3 big [128, 512]
  vector ops per group of 16 bands (copy / relu / add).
- depth-2/1/0: unpadded [128, 32] fp16 stationaries (K-chunks of each
  node's 256-input block); 4 nodes stack onto one [128, 32] psum tile via
  matmul partition offsets (tile_position col base 0/32/64/96), so the
  per-node bias rides a per-partition [128, 1] column in the
  tensor_scalar drain.
- Activations live as [feature-block-on-partition, (block, batch) on
  free]; each layer's psum output lands exactly in the layout the next
  layer's rhs wants, so no transposes/relayouts anywhere.
- The matmul single-sync-wait constraint is handled baseline-style: all
  input DMAs ride one order-chained queue, all psum drains run on the
  vector engine only, and two dummy matmuls absorb the later DMA queue
  ticks before the matmuls that need them.
"""

import numpy as np

import concourse.bacc as bacc
import concourse.bass as bass
import concourse.mybir as mybir
import concourse.tile as tile
from bass_rust import add_dep_helper
from concourse.bass_utils import run_bass_kernel_spmd

NCORES = 8
EPS = 1e-5
F32 = mybir.dt.float32
F16 = mybir.dt.float16
AF = mybir.ActivationFunctionType
ALU = mybir.AluOpType

# blob1a [33, N1A] fp16: xt tiles + w3 groups 0-3
C_XT = 0            # tile t at cols 32t (rows 0-31 = x.T slice, row 32 = ones)
C_W3A = 4096        # group g(0-3), branch br, band i: (g*2+br)*2048 + 128*i
N1A = 4096 + 4 * 4096
# blob1b [33, N1B] fp16: w3 groups 4-7
N1B = 4 * 4096
# blob2 [128, N2] fp16: w2/w1/w0 stationaries
C_W2 = 0            # (G, br, j, c): ((G*2+br)*8 + j*2 + c)*32    [16 G]
C_W1 = 8192         # (G2, br, j, c): same layout                 [2 G2]
C_W0 = 9216         # (br, c): (br*2+c)*32
N2 = 9344
# blob3 [128, N3] fp32: drain bias columns + BN/out params
C_B2 = 0            # cols G*2+br: [bm2' | bl2] per psum group [32]
C_B1 = 32           # cols G2*2+br [4]
C_B0 = 36           # cols br (rows 0-31) [2]
C_B2D = 38          # direct-mode (1-g)*bl2 cols [16]
C_B1D = 54          # [2]
C_B0D = 56          # [1]
C_AUG = 57          # [33, 2] = [Wout.T ; bout]
C_BN = 59           # col 59 = gamma', col 60 = beta (rows 0-31)
N3 = 64


def _extract_blocks(w, n, rows, cols):
    """Diagonal blocks of block-diag matrix w: out[i] = w[i*rows:(i+1)*rows, i*cols:(i+1)*cols]."""
    s0, s1 = w.strides
    return np.lib.stride_tricks.as_strided(
        w, (n, rows, cols), (rows * s0 + cols * s1, s0, s1)
    ).copy()


def _build_module(scaled: bool, g: float, debug: bool = False) -> bass.Bass:
    nc = bacc.Bacc(num_devices=NCORES)

    in1a_d = nc.dram_tensor("in1a", [33, N1A], F16, kind="ExternalInput")
    in1b_d = nc.dram_tensor("in1b", [33, N1B], F16, kind="ExternalInput")
    in3_d = nc.dram_tensor("in3", [128, N3], F32, kind="ExternalInput")
    in2_d = nc.dram_tensor("in2", [128, N2], F16, kind="ExternalInput")
    out_d = nc.dram_tensor("out", [32, 2], F32, kind="ExternalOutput")
    dbg_d = {}
    if debug:
        for nm, shp in [("d_u3", [128, 4096]), ("d_xl3", [128, 4096]),
                        ("d_u2", [128, 512]), ("d_xl2", [128, 512]),
                        ("d_u1", [128, 64]), ("d_xl1", [128, 64]),
                        ("d_u0", [32, 32])]:
            dbg_d[nm] = nc.dram_tensor(nm, shp, F32, kind="ExternalOutput")

    eps_c = EPS / (1.0 - g) ** 2 if scaled else EPS
    sub_combine = (not scaled) and g < 0.0

    with tile.TileContext(nc) as tc:
        with (
            tc.tile_pool(name="weights", bufs=1) as wp,
            tc.tile_pool(name="acts", bufs=1) as acp,
            tc.tile_pool(name="scratch", bufs=2) as sp,
            tc.tile_pool(name="small", bufs=2) as smp,
            tc.tile_pool(name="psL", bufs=4, space="PSUM") as pL,
            tc.tile_pool(name="psS", bufs=3, space="PSUM") as pS,
            tc.tile_pool(name="psD", bufs=1, space="PSUM") as pD,
        ):
            in1a = wp.tile([33, N1A], F16, name="in1a_sb")
            dma1a = nc.gpsimd.dma_start(in1a[:, :], in1a_d[:, :])
            in1b = wp.tile([33, N1B], F16, name="in1b_sb")
            dma1b = nc.gpsimd.dma_start(in1b[:, :], in1b_d[:, :])
            in3 = wp.tile([128, N3], F32, name="in3_sb")
            dma3 = nc.gpsimd.dma_start(in3[:, :], in3_d[:, :])
            in2 = wp.tile([128, N2], F16, name="in2_sb")
            dma2 = nc.gpsimd.dma_start(in2[:, :], in2_d[:, :])
            add_dep_helper(dma1b.ins, dma1a.ins, False, "queue order 1a->1b")
            add_dep_helper(dma3.ins, dma1b.ins, False, "queue order 1b->3")
            add_dep_helper(dma2.ins, dma3.ins, False, "queue order 3->2")

            def xt_t(t):
                return in1a[:, C_XT + 32 * t : C_XT + 32 * (t + 1)]

            def w3ap(gg, br, i):
                if gg < 4:
                    o = C_W3A + (gg * 2 + br) * 2048 + 128 * i
                    return in1a[:, o : o + 128]
                o = ((gg - 4) * 2 + br) * 2048 + 128 * i
                return in1b[:, o : o + 128]

            def w2ap(G, br, j, c):
                o = C_W2 + ((G * 2 + br) * 8 + j * 2 + c) * 32
                return in2[:, o : o + 32]

            def w1ap(G2, br, j, c):
                o = C_W1 + ((G2 * 2 + br) * 8 + j * 2 + c) * 32
                return in2[:, o : o + 32]

            def w0ap(br, c):
                o = C_W0 + (br * 2 + c) * 32
                return in2[:, o : o + 32]

            def b2c(G, br):
                return in3[:, C_B2 + 2 * G + br : C_B2 + 2 * G + br + 1]

            def b1c(G2, br):
                return in3[:, C_B1 + 2 * G2 + br : C_B1 + 2 * G2 + br + 1]

            def b0c(br):
                return in3[0:32, C_B0 + br : C_B0 + br + 1]

            # persistent activations: [feature-in-block on partition, (block, batch) free]
            u3 = acp.tile([128, 4096], F16, name="u3_sb")
            xl3 = acp.tile([128, 4096], F16, name="xl3_sb")
            u2 = acp.tile([128, 512], F16, name="u2_sb")
            xl2 = acp.tile([128, 512], F16, name="xl2_sb")
            u1 = acp.tile([128, 64], F16, name="u1_sb")
            xl1 = acp.tile([128, 64], F16, name="xl1_sb")

            # warm the scalar engine's activation table early (Sqrt is the
            # only scalar op; the one-time table load is ~1.3us)
            eps_t = wp.tile([32, 1], F32, name="eps_t")
            nc.vector.memset(eps_t[:, :], eps_c)
            warm = smp.tile([32, 1], F32, name="warm", tag="warm")
            nc.scalar.activation(warm[:, :], eps_t[:, :], AF.Sqrt, bias=eps_t[:, :],
                                 scale=1.0)

            # ---- depth-3: 8 groups x (16 m-bands + 16 l-bands) ----
            for gg in range(8):
                if gg == 4:
                    # absorb blob1b's queue tick onto PE before its first use
                    psd2 = pD.tile([32, 2], F32, name="psd2", tag="psd")
                    nc.tensor.matmul(
                        psd2[:, :], lhsT=in1b[0:32, 0:32], rhs=in1b[0:32, 0:2],
                        start=True, stop=True,
                    )
                bm = pL.tile([128, 512], F32, name="bm3", tag="psL")
                bl = pL.tile([128, 512], F32, name="bl3", tag="psL")
                for i in range(16):
                    t = 16 * gg + i
                    nc.tensor.matmul(
                        bm[:, 32 * i : 32 * (i + 1)], lhsT=w3ap(gg, 0, i),
                        rhs=xt_t(t), start=True, stop=True,
                    )
                for i in range(16):
                    t = 16 * gg + i
                    nc.tensor.matmul(
                        bl[:, 32 * i : 32 * (i + 1)], lhsT=w3ap(gg, 1, i),
                        rhs=xt_t(t), start=True, stop=True,
                    )
                sl = slice(512 * gg, 512 * (gg + 1))
                xl3s = xl3[:, sl]
                nc.vector.tensor_copy(xl3s, bl[:, :])
                hm = sp.tile([128, 512], F16, name="hm3", tag="hm3")
                nc.vector.tensor_scalar_max(hm[:, :], bm[:, :], 0.0)
                if scaled:
                    nc.vector.tensor_add(u3[:, sl], hm[:, :], xl3s)
                else:
                    xlc = sp.tile([128, 512], F16, name="xlc3", tag="xlc3")
                    nc.vector.tensor_scalar_mul(xlc[:, :], bl[:, :], 1.0 - g)
                    if sub_combine:
                        nc.vector.tensor_sub(u3[:, sl], xlc[:, :], hm[:, :])
                    else:
                        nc.vector.tensor_add(u3[:, sl], hm[:, :], xlc[:, :])

            # absorb blob3+blob2 queue ticks onto PE before depth-2
            psd1 = pD.tile([32, 2], F32, name="psd1", tag="psd")
            nc.tensor.matmul(
                psd1[:, :], lhsT=in2[0:32, 0:32], rhs=in2[0:32, 0:2],
                start=True, stop=True,
            )

            def drain_group(psm, psl, bmcol, blcol, blccol, udst, xldst, hmtag, rows):
                """Bias-add + relu + gate-combine drains for one psum pair."""
                hm = smp.tile([rows, 32], F16, name=hmtag, tag=hmtag)
                nc.vector.tensor_scalar(
                    hm[:, :], psm, bmcol, 0.0, op0=ALU.add, op1=ALU.max
                )
                nc.vector.tensor_scalar(xldst, psl, blcol, None, op0=ALU.add)
                if scaled:
                    nc.vector.tensor_add(udst, hm[:, :], xldst)
                else:
                    xlc = smp.tile([rows, 32], F16, name=hmtag + "c", tag=hmtag + "c")
                    nc.vector.tensor_scalar(
                        xlc[:, :], psl, 1.0 - g, blccol, op0=ALU.mult, op1=ALU.add
                    )
                    if sub_combine:
                        nc.vector.tensor_sub(udst, xlc[:, :], hm[:, :])
                    else:
                        nc.vector.tensor_add(udst, hm[:, :], xlc[:, :])

            # ---- depth-2: 16 psum groups of 4 nodes ----
            for G in range(16):
                psm = pS.tile([128, 32], F32, name="ps2m", tag="psS")
                psl = pS.tile([128, 32], F32, name="ps2l", tag="psS")
                for j in range(4):
                    n = 4 * G + j
                    for c in range(2):
                        tau = 2 * n + c
                        nc.tensor.matmul(
                            psm[32 * j : 32 * (j + 1), :], lhsT=w2ap(G, 0, j, c),
                            rhs=u3[:, 32 * tau : 32 * (tau + 1)],
                            start=(c == 0), stop=(c == 1),
                            tile_position=(0, 32 * j),
                        )
                for j in range(4):
                    n = 4 * G + j
                    for c in range(2):
                        tau = 2 * n + c
                        nc.tensor.matmul(
                            psl[32 * j : 32 * (j + 1), :], lhsT=w2ap(G, 1, j, c),
                            rhs=xl3[:, 32 * tau : 32 * (tau + 1)],
                            start=(c == 0), stop=(c == 1),
                            tile_position=(0, 32 * j),
                        )
                blcc = in3[:, C_B2D + G : C_B2D + G + 1]
                drain_group(psm[:, :], psl[:, :], b2c(G, 0), b2c(G, 1), blcc,
                            u2[:, 32 * G : 32 * (G + 1)],
                            xl2[:, 32 * G : 32 * (G + 1)], "hm2", 128)

            # ---- depth-1: 2 psum groups of 4 nodes ----
            for G2 in range(2):
                psm = pS.tile([128, 32], F32, name="ps1m", tag="psS")
                psl = pS.tile([128, 32], F32, name="ps1l", tag="psS")
                for j in range(4):
                    n = 4 * G2 + j
                    for c in range(2):
                        sg = 2 * n + c
                        nc.tensor.matmul(
                            psm[32 * j : 32 * (j + 1), :], lhsT=w1ap(G2, 0, j, c),
                            rhs=u2[:, 32 * sg : 32 * (sg + 1)],
                            start=(c == 0), stop=(c == 1),
                            tile_position=(0, 32 * j),
                        )
                for j in range(4):
                    n = 4 * G2 + j
                    for c in range(2):
                        sg = 2 * n + c
                        nc.tensor.matmul(
                            psl[32 * j : 32 * (j + 1), :], lhsT=w1ap(G2, 1, j, c),
                            rhs=xl2[:, 32 * sg : 32 * (sg + 1)],
                            start=(c == 0), stop=(c == 1),
                            tile_position=(0, 32 * j),
                        )
                blcc = in3[:, C_B1D + G2 : C_B1D + G2 + 1]
                drain_group(psm[:, :], psl[:, :], b1c(G2, 0), b1c(G2, 1), blcc,
                            u1[:, 32 * G2 : 32 * (G2 + 1)],
                            xl1[:, 32 * G2 : 32 * (G2 + 1)], "hm1", 128)

            # ---- depth-0 (root): one node, fp32 out for batchnorm ----
            ps0m = pS.tile([32, 32], F32, name="ps0m", tag="psS")
            ps0l = pS.tile([32, 32], F32, name="ps0l", tag="psS")
            for c in range(2):
                nc.tensor.matmul(
                    ps0m[:, :], lhsT=w0ap(0, c), rhs=u1[:, 32 * c : 32 * (c + 1)],
                    start=(c == 0), stop=(c == 1),
                )
            for c in range(2):
                nc.tensor.matmul(
                    ps0l[:, :], lhsT=w0ap(1, c), rhs=xl1[:, 32 * c : 32 * (c + 1)],
                    start=(c == 0), stop=(c == 1),
                )
            hm0 = smp.tile([32, 32], F32, name="hm0", tag="hm0")
            xl0 = smp.tile([32, 32], F32, name="xl0", tag="xl0")
            u0 = acp.tile([32, 32], F32, name="u0")
            nc.vector.tensor_scalar(
                hm0[:, :], ps0m[:, :], b0c(0), 0.0, op0=ALU.add, op1=ALU.max
            )
            nc.vector.tensor_scalar(xl0[:, :], ps0l[:, :], b0c(1), None, op0=ALU.add)
            if scaled:
                nc.vector.tensor_add(u0[:, :], hm0[:, :], xl0[:, :])
            else:
                xlc0 = smp.tile([32, 32], F32, name="xlc0", tag="xlc0")
                nc.vector.tensor_scalar(
                    xlc0[:, :], ps0l[:, :], 1.0 - g,
                    in3[0:32, C_B0D : C_B0D + 1], op0=ALU.mult, op1=ALU.add,
                )
                if sub_combine:
                    nc.vector.tensor_sub(u0[:, :], xlc0[:, :], hm0[:, :])
                else:
                    nc.vector.tensor_add(u0[:, :], hm0[:, :], xlc0[:, :])

            # ---- batchnorm over the batch (free) axis ----
            stats = smp.tile([32, 6], F32, name="stats", tag="stats")
            mv = smp.tile([32, 2], F32, name="mv", tag="mv")
            nc.vector.bn_stats(stats[:, :], u0[:, :])
            nc.vector.bn_aggr(mv[:, :], stats[:, :])
            sq = smp.tile([32, 1], F32, name="sq", tag="sq")
            nc.scalar.activation(
                sq[:, :], mv[:, 1:2], AF.Sqrt, bias=eps_t[:, :], scale=1.0
            )
            rstd = smp.tile([32, 1], F32, name="rstd", tag="rstd")
            nc.vector.reciprocal(rstd[:, :], sq[:, :])

            aug = acp.tile([33, 32], F32, name="aug")
            nc.vector.memset(aug[32:33, :], 1.0)
            nc.vector.tensor_scalar(
                aug[0:32, :], u0[:, :], mv[:, 0:1], rstd[:, :],
                op0=ALU.subtract, op1=ALU.mult,
            )
            nc.vector.tensor_scalar(
                aug[0:32, :], aug[0:32, :], in3[0:32, C_BN : C_BN + 1],
                in3[0:32, C_BN + 1 : C_BN + 2], op0=ALU.mult, op1=ALU.add,
            )
            pso = pS.tile([32, 2], F32, name="pso", tag="psS")
            nc.tensor.matmul(
                pso[:, :], lhsT=aug[:, :], rhs=in3[0:33, C_AUG : C_AUG + 2],
                start=True, stop=True,
            )
            outt = smp.tile([32, 2], F32, name="outt", tag="outt")
            nc.vector.tensor_copy(outt[:, :], pso[:, :])
            nc.gpsimd.dma_start(out_d[:, :], outt[:, :])
            if debug:
                for src, nm in [(u3[:, :], "d_u3"), (xl3[:, :], "d_xl3"),
                                (u2[:, :], "d_u2"), (xl2[:, :], "d_xl2"),
                                (u1[:, :], "d_u1"), (xl1[:, :], "d_xl1"),
                                (u0[:, :], "d_u0")]:
                    nc.gpsimd.dma_start(dbg_d[nm][:, :], src)

    nc.finalize()
    return nc


_module_cache: dict = {}


def _get_module(scaled: bool, g: float, debug: bool = False) -> bass.Bass:
    key = (scaled, round(float(g), 12), debug)
    if key not in _module_cache:
        _module_cache[key] = _build_module(scaled, g, debug)
    return _module_cache[key]


def _pack_inputs(x, Wm3, bm3, Wl3, bl3, Wm2, bm2, Wl2, bl2, Wm1, bm1, Wl1, bl1,
                 Wm0, bm0, Wl0, bl0, gate, bn_gamma, bn_beta, Wout, bout,
                 scaled, g):
    f = np.float32
    if scaled:
        aW3 = g / (1.0 - g)  # relu-branch weight factor, d3 (raw input basis)
        aW = g               # relu-branch weight factor, d2/d1/d0 (u basis)
        ab = g / (1.0 - g)   # relu-branch bias factor, all layers
        sgn1mg = 1.0 if (1.0 - g) > 0 else -1.0
    else:
        aW3 = aW = ab = abs(g)
        sgn1mg = 1.0

    x = np.asarray(x, f)
    bm3 = np.asarray(bm3, f); bl3 = np.asarray(bl3, f)
    bm2 = np.asarray(bm2, f); bl2 = np.asarray(bl2, f)
    bm1 = np.asarray(bm1, f); bl1 = np.asarray(bl1, f)
    bm0 = np.asarray(bm0, f); bl0 = np.asarray(bl0, f)

    # blob1: xt + w3 band stationaries [33, 128] (rows 0-31 = block.T, row 32 = bias)
    xt = np.empty((33, 4096), f)
    xt[:32] = x.T.reshape(128, 32, 32).transpose(1, 0, 2).reshape(32, 4096)
    xt[32] = 1.0

    bl3m = _extract_blocks(np.asarray(Wm3, f), 128, 128, 32)  # (128, 128m, 32k)
    bl3l = _extract_blocks(np.asarray(Wl3, f), 128, 128, 32)
    S3 = np.zeros((128, 2, 33, 128), f)                       # (band, br, k, m)
    S3[:, 0, :32] = aW3 * bl3m.transpose(0, 2, 1)
    S3[:, 1, :32] = bl3l.transpose(0, 2, 1)
    S3[:, 0, 32] = ab * bm3.reshape(128, 128)
    S3[:, 1, 32] = bl3.reshape(128, 128)
    # (g, br, i, k, m) -> [33, 32768] with col = ((g*2+br)*16 + i)*128 + m
    w3cols = (S3.reshape(8, 16, 2, 33, 128).transpose(3, 0, 2, 1, 4)
              .reshape(33, 32768))

    in1a = np.empty((33, N1A), np.float16)
    in1a[:, :4096] = xt
    in1a[:, 4096:] = w3cols[:, : 4 * 4096]
    in1b = np.ascontiguousarray(w3cols[:, 4 * 4096 :]).astype(np.float16)

    # blob2: w2/w1/w0 [128, 32] k-chunk stationaries + bias columns
    def mid_stationaries(Wm, Wl, nnodes):
        bm_ = _extract_blocks(np.asarray(Wm, f), nnodes, 32, 256)  # (n, 32m, 256k)
        bl_ = _extract_blocks(np.asarray(Wl, f), nnodes, 32, 256)
        Sm = (aW * bm_).reshape(nnodes, 32, 2, 128).transpose(0, 2, 3, 1)
        Sl = bl_.reshape(nnodes, 32, 2, 128).transpose(0, 2, 3, 1)
        # (G, j, br, c, k, m) -> cols ((G*2+br)*8 + j*2 + c)*32 + m
        S = np.stack([Sm.reshape(nnodes // 4, 4, 2, 128, 32),
                      Sl.reshape(nnodes // 4, 4, 2, 128, 32)], axis=2)
        return (S.transpose(4, 0, 2, 1, 3, 5)          # (k, G, br, j, c, m)
                .reshape(128, nnodes * 2 * 2 * 32))

    in2 = np.zeros((128, N2), f)
    in2[:, C_W2 : C_W2 + 8192] = mid_stationaries(Wm2, Wl2, 64)
    in2[:, C_W1 : C_W1 + 1024] = mid_stationaries(Wm1, Wl1, 8)
    Wm0 = np.asarray(Wm0, f); Wl0 = np.asarray(Wl0, f)
    S0 = np.stack([(aW * Wm0).reshape(32, 2, 128).transpose(1, 2, 0),
                   Wl0.reshape(32, 2, 128).transpose(1, 2, 0)], axis=0)
    in2[:, C_W0 : C_W0 + 128] = S0.transpose(2, 0, 1, 3).reshape(128, 128)
    in2 = in2.astype(np.float16)

    in3 = np.zeros((128, N3), f)
    in3[:, C_B2 + 0 : C_B2 + 32 : 2] = ab * bm2.reshape(16, 128).T
    in3[:, C_B2 + 1 : C_B2 + 32 : 2] = bl2.reshape(16, 128).T
    in3[:, C_B1 + 0 : C_B1 + 4 : 2] = ab * bm1.reshape(2, 128).T
    in3[:, C_B1 + 1 : C_B1 + 4 : 2] = bl1.reshape(2, 128).T
    in3[0:32, C_B0 + 0] = ab * bm0
    in3[0:32, C_B0 + 1] = bl0
    in3[:, C_B2D : C_B2D + 16] = (1.0 - g) * bl2.reshape(16, 128).T
    in3[:, C_B1D : C_B1D + 2] = (1.0 - g) * bl1.reshape(2, 128).T
    in3[0:32, C_B0D] = (1.0 - g) * bl0
    in3[:32, C_AUG : C_AUG + 2] = np.asarray(Wout, f).T
    in3[32, C_AUG : C_AUG + 2] = np.asarray(bout, f)
    in3[:32, C_BN] = sgn1mg * np.asarray(bn_gamma, f)
    in3[:32, C_BN + 1] = np.asarray(bn_beta, f)

    im = {"in1a": in1a, "in1b": in1b, "in2": in2, "in3": in3}
    return [im for _ in range(NCORES)]


def kernel(x, Wm3, bm3, Wl3, bl3, Wm2, bm2, Wl2, bl2, Wm1, bm1, Wl1, bl1,
           Wm0, bm0, Wl0, bl0, gate, bn_gamma, bn_beta, Wout, bout,
           _trace=False, _trace_kwargs=None, _debug=False):
    g = float(np.asarray(gate))
    scaled = abs(1.0 - g) > 1e-6 and (g / (1.0 - g)) >= 0.0
    nc = _get_module(scaled, g, _debug)
    in_maps = _pack_inputs(
        x, Wm3, bm3, Wl3, bl3, Wm2, bm2, Wl2, bl2, Wm1, bm1, Wl1, bl1,
        Wm0, bm0, Wl0, bl0, gate, bn_gamma, bn_beta, Wout, bout, scaled, g,
    )
    kwargs = dict(_trace_kwargs or {})
    res = run_bass_kernel_spmd(
        nc, in_maps, core_ids=list(range(NCORES)), trace=_trace, **kwargs
    )
    out = np.asarray(res.results[0]["out"], np.float32)
    if _debug or _trace:
        return out, res
    return out


# revision 35
# speedup vs baseline: 2.1193x; 1.1952x over previous
"""Trainium2 Bass kernel for the MIOSTONE tree model (8-core SPMD).

Strategy (v3: fully replicated, collective-free, DMA/engine-balanced)
---------------------------------------------------------------------
The two big weight matrices are block-diagonal (tree structure), so the
dense 772 MB of weights carry only ~5.6 MB of real data.  Profiling showed
any cross-core scheme pays ~60 us of collective machinery (a ~50 us CC
barrier + a slow 64 KB AllGather), dwarfing the ~15 us of compute — so
every core redundantly computes the full tree from ~5 MB of fp16-packed
weights and core 0's output is returned.  No cross-core traffic.

v3 specifics (from v2 trace analysis):
- All input blobs are [128, N] so all 16 SDMA engines participate
  (a [33, N] blob runs at ~112 GB/s vs ~350 GB/s).  depth-3 x-tiles and
  band stationaries are packed 4-high at partition bases {0,32,64,96}
  via explicit matmul tile_position.
- depth-3 band stationaries are [32, 128] (4 tree nodes' 32x8 diagonal
  blocks); 16 bands accumulate side by side into a [128, 512] PSUM bank.
  The m-branch bias enters via ONE extra matmul per bank: lhsT = the 16
  bands' bias rows [16, 128], rhs = kron(I16, ones(1,32)) — it writes
  bias into the whole bank (start=True), then band matmuls accumulate.
- l-branch biases are folded downstream on the host (bl_acc chains into
  the next layer's m-branch drain bias and the final l drain), so every
  l-branch drain is a pure fp32->fp16 cast that runs on the SCALAR
  engine; relu/combine drains run on the vector engine.
- Engine discipline: each matmul's psum-bank WAR engine equals its
  rhs-producer engine (m-banks: vector, l-banks: scalar), so with the
  two dummy matmuls that absorb DMA queue ticks, every matmul needs at
  most one new sync wait.
- Activations are [feature-in-block on partition, (block, batch) free];
  each layer's psum lands exactly in the next layer's rhs layout — no
  transposes or relayouts anywhere.  fp16 weights/activations (fp32
  PSUM): ~2.7e-3 rel err vs the 2e-2 gate.
"""

import numpy as np

import concourse.bacc as bacc
import concourse.bass as bass
import concourse.mybir as mybir
import concourse.tile as tile
from bass_rust import add_dep_helper
from concourse.bass_utils import run_bass_kernel_spmd

NCORES = 8
EPS = 1e-5
F32 = mybir.dt.float32
F16 = mybir.dt.float16
AF = mybir.ActivationFunctionType
ALU = mybir.AluOpType

# blob1 [128, *] fp16, split for early compute start:
#   blob1a: kron | xt | biasT | w3 clusters 0-31 (groups 0-3)
#   blob1b: w3 clusters 32-63 (groups 4-7)
C_KR = 0            # kron(I16, ones(1,32)) replicated at rows {0,32,64}
C_XT = 512          # x tile t: rows 32*(t%3), cols C_XT + 32*(t//3)   [43 stripes]
C_BT = 1888         # biasT m-bank g: rows 32*(g%3)+(0..16), cols C_BT+128*(g//3)
C_BTL = 2272        # biasT l-bank g (direct mode only): same layout
C_W3 = 2656         # band t, br: rows 32*(t%3), col block 2*(t//3)+br
N1A = 2656 + 22 * 2 * 128   # w3 blocks b3=0..21 (bands 0-65)
N1B = 21 * 2 * 128          # w3 blocks b3=22..42
# blob2 [128, N2] fp16: w2/w1/w0 k-chunk stationaries
C_W2 = 0            # (G, br, j, c): ((G*2+br)*8 + j*2 + c)*32    [16 G]
C_W1 = 8192         # (G2, br, j, c): same layout                 [2 G2]
C_W0 = 9216         # (br, c): (br*2+c)*32
N2 = 9344
# blob3 [128, N3] fp32: drain bias columns + BN/out params
C_B2M = 0           # bm2_eff per psum group [16]
C_B1M = 16          # bm1_eff [2]
C_B0 = 18           # col 0 = bm0_eff, col 1 = l drain bias (rows 0-31)
C_B2D = 20          # direct-mode (1-g)*bl2 [16]
C_B1D = 36          # [2]
C_B2T = 38          # direct-mode plain bl2 (l-chain drain) [16]
C_B1T = 54          # [2]
C_AUG = 56          # [33, 2] = [Wout.T ; bout]
C_BN = 58           # col 58 = gamma', col 59 = beta (rows 0-31)
N3 = 60


def _d3_banks():
    """9 psum banks, each holding only bands with one partition-row residue
    (mixing row bases within one PSUM bank hangs the device).  Ordered so
    the blob1a/1b column split is crossed as late as possible."""
    banks = []
    start = 0
    for q in range(3):
        for r in range(3):
            size = 16 if q < 2 else (11, 11, 10)[r]
            ts = [3 * (16 * q + j) + r for j in range(size)]
            banks.append((r, q, size, start, ts))
            start += size
    return banks


_D3_BANKS = _d3_banks()
_D3_POS = {t: start + j
           for (r, q, size, start, ts) in _D3_BANKS
           for j, t in enumerate(ts)}


def _extract_blocks(w, n, rows, cols):
    """Diagonal blocks of block-diag matrix w: out[i] = w[i*rows:(i+1)*rows, i*cols:(i+1)*cols]."""
    s0, s1 = w.strides
    return np.lib.stride_tricks.as_strided(
        w, (n, rows, cols), (rows * s0 + cols * s1, s0, s1)
    ).copy()


def _build_module(scaled: bool, g: float, debug: bool = False) -> bass.Bass:
    nc = bacc.Bacc(num_devices=NCORES)

    in1a_d = nc.dram_tensor("in1a", [128, N1A], F16, kind="ExternalInput")
    in1b_d = nc.dram_tensor("in1b", [128, N1B], F16, kind="ExternalInput")
    in3_d = nc.dram_tensor("in3", [128, N3], F32, kind="ExternalInput")
    in2_d = nc.dram_tensor("in2", [128, N2], F16, kind="ExternalInput")
    out_d = nc.dram_tensor("out", [32, 2], F32, kind="ExternalOutput")
    dbg_d = {}
    if debug:
        for nm, shp in [("d_u3", [128, 4096]), ("d_xl3", [128, 4096]),
                        ("d_u2", [128, 512]), ("d_xl2", [128, 512]),
                        ("d_u1", [128, 64]), ("d_xl1", [128, 64]),
                        ("d_u0", [32, 32])]:
            dbg_d[nm] = nc.dram_tensor(nm, shp, F32, kind="ExternalOutput")

    eps_c = EPS / (1.0 - g) ** 2 if scaled else EPS
    sub_combine = (not scaled) and g < 0.0

    with tile.TileContext(nc) as tc:
        with (
            tc.tile_pool(name="weights", bufs=1) as wp,
            tc.tile_pool(name="acts", bufs=1) as acp,
            tc.tile_pool(name="scratch", bufs=2) as sp,
            tc.tile_pool(name="small", bufs=2) as smp,
            tc.tile_pool(name="psM", bufs=2, space="PSUM") as pM,
            tc.tile_pool(name="psL", bufs=2, space="PSUM") as pL,
            tc.tile_pool(name="psSm", bufs=2, space="PSUM") as pSm,
            tc.tile_pool(name="psSl", bufs=2, space="PSUM") as pSl,
        ):
            in1a = wp.tile([128, N1A], F16, name="in1a_sb")
            dma1a = nc.gpsimd.dma_start(in1a[:, :], in1a_d[:, :])
            in1b = wp.tile([128, N1B], F16, name="in1b_sb")
            dma1b = nc.gpsimd.dma_start(in1b[:, :], in1b_d[:, :])
            in3 = wp.tile([128, N3], F32, name="in3_sb")
            dma3 = nc.gpsimd.dma_start(in3[:, :], in3_d[:, :])
            in2 = wp.tile([128, N2], F16, name="in2_sb")
            dma2 = nc.gpsimd.dma_start(in2[:, :], in2_d[:, :])
            add_dep_helper(dma1b.ins, dma1a.ins, False, "queue order 1a->1b")
            add_dep_helper(dma3.ins, dma1b.ins, False, "queue order 1b->3")
            add_dep_helper(dma2.ins, dma3.ins, False, "queue order 3->2")

            def xt_t(t):
                r, q = t % 3, t // 3
                return in1a[32 * r : 32 * r + 32, C_XT + 32 * q : C_XT + 32 * q + 32]

            def w3ap(t, br):
                r, b3 = t % 3, t // 3
                blk = 2 * b3 + br
                if b3 < 22:
                    return in1a[32 * r : 32 * r + 32,
                                C_W3 + 128 * blk : C_W3 + 128 * (blk + 1)]
                o = 128 * (blk - 44)
                return in1b[32 * r : 32 * r + 32, o : o + 128]

            def btap(r, q, size, base=C_BT):
                return in1a[32 * r : 32 * r + size,
                            base + 128 * q : base + 128 * q + 128]

            def krap(r, size):
                return in1a[32 * r : 32 * r + size, C_KR : C_KR + 32 * size]

            def w2ap(G, br, j, c):
                o = C_W2 + ((G * 2 + br) * 8 + j * 2 + c) * 32
                return in2[:, o : o + 32]

            def w1ap(G2, br, j, c):
                o = C_W1 + ((G2 * 2 + br) * 8 + j * 2 + c) * 32
                return in2[:, o : o + 32]

            def w0ap(br, c):
                o = C_W0 + (br * 2 + c) * 32
                return in2[:, o : o + 32]

            # persistent activations: [feature-in-block on partition, (block, batch) free]
            u3 = acp.tile([128, 4096], F16, name="u3_sb")
            xl3 = acp.tile([128, 4096], F16, name="xl3_sb")
            u2 = acp.tile([128, 512], F16, name="u2_sb")
            xl2 = acp.tile([128, 512], F16, name="xl2_sb")
            u1 = acp.tile([128, 64], F16, name="u1_sb")
            xl1 = acp.tile([128, 64], F16, name="xl1_sb")

            # warm the scalar engine's activation table early (one-time ~1.3us)
            eps_t = wp.tile([32, 1], F32, name="eps_t")
            nc.vector.memset(eps_t[:, :], eps_c)
            warm = smp.tile([32, 1], F32, name="warm", tag="warm")
            nc.scalar.activation(warm[:, :], eps_t[:, :], AF.Sqrt, bias=eps_t[:, :],
                                 scale=1.0)

            # ---- depth-3: 9 pure-residue banks (bias-MM + band MMs) ----
            for bk, (r, q, size, start, ts) in enumerate(_D3_BANKS):
                if bk == 3:
                    # absorb blob1b's queue tick onto PE before its first use
                    psd2 = pSm.tile([32, 2], F32, name="psd2", tag="m")
                    nc.tensor.matmul(
                        psd2[:, :], lhsT=in1b[0:32, 0:32], rhs=in1b[0:32, 0:2],
                        start=True, stop=True,
                    )
                ns = 32 * size
                bm = pM.tile([128, 512], F32, name="bm3", tag="psM")
                bl = pL.tile([128, 512], F32, name="bl3", tag="psL")
                nc.tensor.matmul(
                    bm[:, 0:ns], lhsT=btap(r, q, size), rhs=krap(r, size),
                    start=True, stop=False, tile_position=(32 * r, 0),
                    skip_group_check=True,
                )
                for j, t in enumerate(ts):
                    nc.tensor.matmul(
                        bm[:, 32 * j : 32 * (j + 1)], lhsT=w3ap(t, 0),
                        rhs=xt_t(t), start=False, stop=True,
                        tile_position=(32 * r, 0), skip_group_check=True,
                    )
                if not scaled:
                    # direct mode: psl must carry bl3 (true x_linear chain)
                    nc.tensor.matmul(
                        bl[:, 0:ns], lhsT=btap(r, q, size, C_BTL),
                        rhs=krap(r, size),
                        start=True, stop=False, tile_position=(32 * r, 0),
                        skip_group_check=True,
                    )
                for j, t in enumerate(ts):
                    nc.tensor.matmul(
                        bl[:, 32 * j : 32 * (j + 1)], lhsT=w3ap(t, 1),
                        rhs=xt_t(t), start=scaled, stop=True,
                        tile_position=(32 * r, 0),
                        skip_group_check=not scaled,
                    )
                sl = slice(32 * start, 32 * (start + size))
                xl3s = xl3[:, sl]
                # l: pure cast (bias deferred downstream)
                nc.vector.tensor_copy(xl3s, bl[:, 0:ns])
                hm = sp.tile([128, 512], F16, name="hm3", tag="hm3")
                nc.vector.tensor_scalar_max(hm[:, 0:ns], bm[:, 0:ns], 0.0)
                if scaled:
                    nc.vector.tensor_add(u3[:, sl], hm[:, 0:ns], xl3s)
                else:
                    xlc = sp.tile([128, 512], F16, name="xlc3", tag="xlc3")
                    nc.vector.tensor_scalar_mul(xlc[:, 0:ns], bl[:, 0:ns], 1.0 - g)
                    if sub_combine:
                        nc.vector.tensor_sub(u3[:, sl], xlc[:, 0:ns], hm[:, 0:ns])
                    else:
                        nc.vector.tensor_add(u3[:, sl], hm[:, 0:ns], xlc[:, 0:ns])

            # absorb blob3+blob2 queue ticks onto PE before depth-2
            psd1 = pSm.tile([32, 2], F32, name="psd1", tag="m")
            nc.tensor.matmul(
                psd1[:, :], lhsT=in2[0:32, 0:32], rhs=in2[0:32, 0:2],
                start=True, stop=True,
            )

            def mid_drains(psm, psl, bmcol, udst, xldst, blccol, bltcol, hmtag,
                           rows):
                """m: relu(z+bias) on vector; l: pure cast on scalar; combine on vector."""
                hm = smp.tile([rows, 32], F16, name=hmtag, tag=hmtag)
                nc.vector.tensor_scalar(
                    hm[:, :], psm, bmcol, 0.0, op0=ALU.add, op1=ALU.max
                )
                if scaled:
                    nc.vector.tensor_copy(xldst, psl)
                    nc.vector.tensor_add(udst, hm[:, :], xldst)
                else:
                    # direct mode: true l chain, bias via columns (vector)
                    nc.vector.tensor_scalar(xldst, psl, bltcol, None, op0=ALU.add)
                    xlc = smp.tile([rows, 32], F16, name=hmtag + "c", tag=hmtag + "c")
                    nc.vector.tensor_scalar(
                        xlc[:, :], psl, 1.0 - g, blccol, op0=ALU.mult, op1=ALU.add
                    )
                    if sub_combine:
                        nc.vector.tensor_sub(udst, xlc[:, :], hm[:, :])
                    else:
                        nc.vector.tensor_add(udst, hm[:, :], xlc[:, :])

            # ---- depth-2: 16 psum groups of 4 nodes ----
            for G in range(16):
                psm = pSm.tile([128, 32], F32, name="ps2m", tag="m")
                psl = pSl.tile([128, 32], F32, name="ps2l", tag="l")
                for j in range(4):
                    n = 4 * G + j
                    for c in range(2):
                        tau = _D3_POS[2 * n + c]
                        nc.tensor.matmul(
                            psm[32 * j : 32 * (j + 1), :], lhsT=w2ap(G, 0, j, c),
                            rhs=u3[:, 32 * tau : 32 * (tau + 1)],
                            start=(c == 0), stop=(c == 1),
                            tile_position=(0, 32 * j),
                        )
                for j in range(4):
                    n = 4 * G + j
                    for c in range(2):
                        tau = _D3_POS[2 * n + c]
                        nc.tensor.matmul(
                            psl[32 * j : 32 * (j + 1), :], lhsT=w2ap(G, 1, j, c),
                            rhs=xl3[:, 32 * tau : 32 * (tau + 1)],
                            start=(c == 0), stop=(c == 1),
                            tile_position=(0, 32 * j),
                        )
                mid_drains(psm[:, :], psl[:, :],
                           in3[:, C_B2M + G : C_B2M + G + 1],
                           u2[:, 32 * G : 32 * (G + 1)],
                           xl2[:, 32 * G : 32 * (G + 1)],
                           in3[:, C_B2D + G : C_B2D + G + 1],
                           in3[:, C_B2T + G : C_B2T + G + 1], "hm2", 128)

            # ---- depth-1: 2 psum groups of 4 nodes ----
            for G2 in range(2):
                psm = pSm.tile([128, 32], F32, name="ps1m", tag="m")
                psl = pSl.tile([128, 32], F32, name="ps1l", tag="l")
                for j in range(4):
                    n = 4 * G2 + j
                    for c in range(2):
                        sg = 2 * n + c
                        nc.tensor.matmul(
                            psm[32 * j : 32 * (j + 1), :], lhsT=w1ap(G2, 0, j, c),
                            rhs=u2[:, 32 * sg : 32 * (sg + 1)],
                            start=(c == 0), stop=(c == 1),
                            tile_position=(0, 32 * j),
                        )
                for j in range(4):
                    n = 4 * G2 + j
                    for c in range(2):
                        sg = 2 * n + c
                        nc.tensor.matmul(
                            psl[32 * j : 32 * (j + 1), :], lhsT=w1ap(G2, 1, j, c),
                            rhs=xl2[:, 32 * sg : 32 * (sg + 1)],
                            start=(c == 0), stop=(c == 1),
                            tile_position=(0, 32 * j),
                        )
                mid_drains(psm[:, :], psl[:, :],
                           in3[:, C_B1M + G2 : C_B1M + G2 + 1],
                           u1[:, 32 * G2 : 32 * (G2 + 1)],
                           xl1[:, 32 * G2 : 32 * (G2 + 1)],
                           in3[:, C_B1D + G2 : C_B1D + G2 + 1],
                           in3[:, C_B1T + G2 : C_B1T + G2 + 1], "hm1", 128)

            # ---- depth-0 (root): one node, fp32 out for batchnorm ----
            ps0m = pSm.tile([32, 32], F32, name="ps0m", tag="m")
            ps0l = pSl.tile([32, 32], F32, name="ps0l", tag="l")
            for c in range(2):
                nc.tensor.matmul(
                    ps0m[:, :], lhsT=w0ap(0, c), rhs=u1[:, 32 * c : 32 * (c + 1)],
                    start=(c == 0), stop=(c == 1),
                )
            for c in range(2):
                nc.tensor.matmul(
                    ps0l[:, :], lhsT=w0ap(1, c), rhs=xl1[:, 32 * c : 32 * (c + 1)],
                    start=(c == 0), stop=(c == 1),
                )
            hm0 = smp.tile([32, 32], F32, name="hm0", tag="hm0")
            xl0 = smp.tile([32, 32], F32, name="xl0", tag="xl0")
            u0 = acp.tile([32, 32], F32, name="u0")
            nc.vector.tensor_scalar(
                hm0[:, :], ps0m[:, :], in3[0:32, C_B0 : C_B0 + 1], 0.0,
                op0=ALU.add, op1=ALU.max,
            )
            if scaled:
                nc.vector.tensor_scalar(
                    xl0[:, :], ps0l[:, :], in3[0:32, C_B0 + 1 : C_B0 + 2], None,
                    op0=ALU.add,
                )
                nc.vector.tensor_add(u0[:, :], hm0[:, :], xl0[:, :])
            else:
                nc.vector.tensor_scalar(
                    xl0[:, :], ps0l[:, :], 1.0 - g,
                    in3[0:32, C_B0 + 1 : C_B0 + 2], op0=ALU.mult, op1=ALU.add,
                )
                if sub_combine:
                    nc.vector.tensor_sub(u0[:, :], xl0[:, :], hm0[:, :])
                else:
                    nc.vector.tensor_add(u0[:, :], hm0[:, :], xl0[:, :])

            # ---- batchnorm over the batch (free) axis ----
            stats = smp.tile([32, 6], F32, name="stats", tag="stats")
            mv = smp.tile([32, 2], F32, name="mv", tag="mv")
            nc.vector.bn_stats(stats[:, :], u0[:, :])
            nc.vector.bn_aggr(mv[:, :], stats[:, :])
            sq = smp.tile([32, 1], F32, name="sq", tag="sq")
            nc.scalar.activation(
                sq[:, :], mv[:, 1:2], AF.Sqrt, bias=eps_t[:, :], scale=1.0
            )
            rstd = smp.tile([32, 1], F32, name="rstd", tag="rstd")
            nc.vector.reciprocal(rstd[:, :], sq[:, :])

            aug = acp.tile([33, 32], F32, name="aug")
            nc.vector.memset(aug[32:33, :], 1.0)
            nc.vector.tensor_scalar(
                aug[0:32, :], u0[:, :], mv[:, 0:1], rstd[:, :],
                op0=ALU.subtract, op1=ALU.mult,
            )
            nc.vector.tensor_scalar(
                aug[0:32, :], aug[0:32, :], in3[0:32, C_BN : C_BN + 1],
                in3[0:32, C_BN + 1 : C_BN + 2], op0=ALU.mult, op1=ALU.add,
            )
            pso = pSm.tile([32, 2], F32, name="pso", tag="m")
            nc.tensor.matmul(
                pso[:, :], lhsT=aug[:, :], rhs=in3[0:33, C_AUG : C_AUG + 2],
                start=True, stop=True,
            )
            outt = smp.tile([32, 2], F32, name="outt", tag="outt")
            nc.vector.tensor_copy(outt[:, :], pso[:, :])
            nc.gpsimd.dma_start(out_d[:, :], outt[:, :])
            if debug:
                for src, nm in [(u3[:, :], "d_u3"), (xl3[:, :], "d_xl3"),
                                (u2[:, :], "d_u2"), (xl2[:, :], "d_xl2"),
                                (u1[:, :], "d_u1"), (xl1[:, :], "d_xl1"),
                                (u0[:, :], "d_u0")]:
                    nc.gpsimd.dma_start(dbg_d[nm][:, :], src)

    nc.finalize()
    return nc


_module_cache: dict = {}


def _get_module(scaled: bool, g: float, debug: bool = False) -> bass.Bass:
    key = (scaled, round(float(g), 12), debug)
    if key not in _module_cache:
        _module_cache[key] = _build_module(scaled, g, debug)
    return _module_cache[key]


def _pack_inputs(x, Wm3, bm3, Wl3, bl3, Wm2, bm2, Wl2, bl2, Wm1, bm1, Wl1, bl1,
                 Wm0, bm0, Wl0, bl0, gate, bn_gamma, bn_beta, Wout, bout,
                 scaled, g):
    f = np.float32
    if scaled:
        aW3 = g / (1.0 - g)  # relu-branch weight factor, d3 (raw input basis)
        aW = g               # relu-branch weight factor, d2/d1/d0 (u basis)
        ab = g / (1.0 - g)   # relu-branch bias factor, all layers
        sgn1mg = 1.0 if (1.0 - g) > 0 else -1.0
    else:
        aW3 = aW = ab = abs(g)
        sgn1mg = 1.0

    x = np.asarray(x, f)
    bm3 = np.asarray(bm3, f); bl3 = np.asarray(bl3, f)
    bm2 = np.asarray(bm2, f); bl2 = np.asarray(bl2, f)
    bm1 = np.asarray(bm1, f); bl1 = np.asarray(bl1, f)
    bm0 = np.asarray(bm0, f); bl0 = np.asarray(bl0, f)
    Wm0 = np.asarray(Wm0, f); Wl0 = np.asarray(Wl0, f)

    bl3m = _extract_blocks(np.asarray(Wm3, f), 128, 128, 32)  # (band, 128m, 32k)
    bl3l = _extract_blocks(np.asarray(Wl3, f), 128, 128, 32)
    bl2m = _extract_blocks(np.asarray(Wm2, f), 64, 32, 256)   # (n, 32m, 256k)
    bl2l = _extract_blocks(np.asarray(Wl2, f), 64, 32, 256)
    bl1m = _extract_blocks(np.asarray(Wm1, f), 8, 32, 256)
    bl1l = _extract_blocks(np.asarray(Wl1, f), 8, 32, 256)

    # l-branch bias accumulators (bias deferred through the bias-free l chain)
    bl2_acc = bl2 + np.einsum("nij,nj->ni", bl2l, bl3.reshape(64, 256)).reshape(-1)
    bl1_acc = bl1 + np.einsum("nij,nj->ni", bl1l, bl2_acc.reshape(8, 256)).reshape(-1)
    bl0_acc = bl0 + Wl0 @ bl1_acc
    if scaled:
        # m-branch drain biases absorb aW * Wm @ bl_acc of the previous level
        bm2_eff = ab * bm2 + aW * np.einsum(
            "nij,nj->ni", bl2m, bl3.reshape(64, 256)).reshape(-1)
        bm1_eff = ab * bm1 + aW * np.einsum(
            "nij,nj->ni", bl1m, bl2_acc.reshape(8, 256)).reshape(-1)
        bm0_eff = ab * bm0 + aW * (Wm0 @ bl1_acc)
        b0l_col = bl0_acc
    else:
        # direct mode stores true x / x_l (biases applied at each level)
        bm2_eff = ab * bm2
        bm1_eff = ab * bm1
        bm0_eff = ab * bm0
        b0l_col = (1.0 - g) * bl0

    # ---- blob1: kron | xt | biasT | w3 (4-high at bases 0/32/64/96) ----
    kron = np.kron(np.eye(16, dtype=f), np.ones((1, 32), f))   # [16, 512]
    tiles = x.T.reshape(128, 32, 32)                           # tile t = x[:,32t:+32].T
    tpad = np.zeros((129, 32, 32), f); tpad[:128] = tiles
    xt = tpad.reshape(43, 3, 32, 32).transpose(1, 2, 0, 3).reshape(96, 43 * 32)
    bT3 = (ab * bm3).reshape(128, 128)                         # [band, m]
    S3 = np.stack([aW3 * bl3m.transpose(0, 2, 1),
                   bl3l.transpose(0, 2, 1)], axis=1)           # (band, br, 32k, 128m)
    S3p = np.zeros((129, 2, 32, 128), f); S3p[:128] = S3
    # col block 2*b3+br holds bands 3*b3+e at rows 32e
    w3cols = (S3p.reshape(43, 3, 2, 32, 128)                   # (b3, e, br, k, m)
              .transpose(1, 3, 0, 2, 4)                        # (e, k, b3, br, m)
              .reshape(96, 43 * 2 * 128))

    blob1 = np.zeros((128, N1A + N1B), f)
    for r in range(3):
        blob1[32 * r : 32 * r + 16, C_KR : C_KR + 512] = kron
    blob1[0:96, C_XT : C_XT + 43 * 32] = xt
    bTl3 = bl3.reshape(128, 128)
    for (rr, q, size, start, ts) in _D3_BANKS:
        blob1[32 * rr : 32 * rr + size,
              C_BT + 128 * q : C_BT + 128 * q + 128] = bT3[ts]
        if not scaled:
            blob1[32 * rr : 32 * rr + size,
                  C_BTL + 128 * q : C_BTL + 128 * q + 128] = bTl3[ts]
    blob1[0:96, C_W3 :] = w3cols
    in1a = blob1[:, :N1A].astype(np.float16)
    in1b = np.ascontiguousarray(blob1[:, N1A:]).astype(np.float16)

    # ---- blob2: w2/w1/w0 [128, 32] k-chunk stationaries ----
    def mid_stationaries(bm_, bl_, nnodes):
        Sm = (aW * bm_).reshape(nnodes, 32, 2, 128).transpose(0, 2, 3, 1)
        Sl = bl_.reshape(nnodes, 32, 2, 128).transpose(0, 2, 3, 1)
        S = np.stack([Sm.reshape(nnodes // 4, 4, 2, 128, 32),
                      Sl.reshape(nnodes // 4, 4, 2, 128, 32)], axis=2)
        return (S.transpose(4, 0, 2, 1, 3, 5)                  # (k, G, br, j, c, m)
                .reshape(128, nnodes * 2 * 2 * 32))

    in2 = np.zeros((128, N2), f)
    in2[:, C_W2 : C_W2 + 8192] = mid_stationaries(bl2m, bl2l, 64)
    in2[:, C_W1 : C_W1 + 1024] = mid_stationaries(bl1m, bl1l, 8)
    S0 = np.stack([(aW * Wm0).reshape(32, 2, 128).transpose(1, 2, 0),
                   Wl0.reshape(32, 2, 128).transpose(1, 2, 0)], axis=0)
    in2[:, C_W0 : C_W0 + 128] = S0.transpose(2, 0, 1, 3).reshape(128, 128)
    in2 = in2.astype(np.float16)

    # ---- blob3: fp32 bias columns + BN/out ----
    in3 = np.zeros((128, N3), f)
    in3[:, C_B2M : C_B2M + 16] = bm2_eff.reshape(16, 128).T
    in3[:, C_B1M : C_B1M + 2] = bm1_eff.reshape(2, 128).T
    in3[0:32, C_B0] = bm0_eff
    in3[0:32, C_B0 + 1] = b0l_col
    in3[:, C_B2D : C_B2D + 16] = (1.0 - g) * bl2.reshape(16, 128).T
    in3[:, C_B1D : C_B1D + 2] = (1.0 - g) * bl1.reshape(2, 128).T
    in3[:, C_B2T : C_B2T + 16] = bl2.reshape(16, 128).T
    in3[:, C_B1T : C_B1T + 2] = bl1.reshape(2, 128).T
    in3[:32, C_AUG : C_AUG + 2] = np.asarray(Wout, f).T
    in3[32, C_AUG : C_AUG + 2] = np.asarray(bout, f)
    in3[:32, C_BN] = sgn1mg * np.asarray(bn_gamma, f)
    in3[:32, C_BN + 1] = np.asarray(bn_beta, f)

    im = {"in1a": in1a, "in1b": in1b, "in2": in2, "in3": in3}
    return [im for _ in range(NCORES)]


def kernel(x, Wm3, bm3, Wl3, bl3, Wm2, bm2, Wl2, bl2, Wm1, bm1, Wl1, bl1,
           Wm0, bm0, Wl0, bl0, gate, bn_gamma, bn_beta, Wout, bout,
           _trace=False, _trace_kwargs=None, _debug=False):
    g = float(np.asarray(gate))
    scaled = abs(1.0 - g) > 1e-6 and (g / (1.0 - g)) >= 0.0
    nc = _get_module(scaled, g, _debug)
    in_maps = _pack_inputs(
        x, Wm3, bm3, Wl3, bl3, Wm2, bm2, Wl2, bl2, Wm1, bm1, Wl1, bl1,
        Wm0, bm0, Wl0, bl0, gate, bn_gamma, bn_beta, Wout, bout, scaled, g,
    )
    kwargs = dict(_trace_kwargs or {})
    res = run_bass_kernel_spmd(
        nc, in_maps, core_ids=list(range(NCORES)), trace=_trace, **kwargs
    )
    out = np.asarray(res.results[0]["out"], np.float32)
    if _debug or _trace:
        return out, res
    return out


# revision 36
# speedup vs baseline: 2.2828x; 1.0772x over previous
"""Trainium2 Bass kernel for the MIOSTONE tree model (8-core SPMD).

Strategy (v3: fully replicated, collective-free, DMA/engine-balanced)
---------------------------------------------------------------------
The two big weight matrices are block-diagonal (tree structure), so the
dense 772 MB of weights carry only ~5.6 MB of real data.  Profiling showed
any cross-core scheme pays ~60 us of collective machinery (a ~50 us CC
barrier + a slow 64 KB AllGather), dwarfing the ~15 us of compute — so
every core redundantly computes the full tree from ~5 MB of fp16-packed
weights and core 0's output is returned.  No cross-core traffic.

v3 specifics (from v2 trace analysis):
- All input blobs are [128, N] so all 16 SDMA engines participate
  (a [33, N] blob runs at ~112 GB/s vs ~350 GB/s).  depth-3 x-tiles and
  band stationaries are packed 4-high at partition bases {0,32,64,96}
  via explicit matmul tile_position.
- depth-3 band stationaries are [32, 128] (4 tree nodes' 32x8 diagonal
  blocks); 16 bands accumulate side by side into a [128, 512] PSUM bank.
  The m-branch bias enters via ONE extra matmul per bank: lhsT = the 16
  bands' bias rows [16, 128], rhs = kron(I16, ones(1,32)) — it writes
  bias into the whole bank (start=True), then band matmuls accumulate.
- l-branch biases are folded downstream on the host (bl_acc chains into
  the next layer's m-branch drain bias and the final l drain), so every
  l-branch drain is a pure fp32->fp16 cast that runs on the SCALAR
  engine; relu/combine drains run on the vector engine.
- Engine discipline: each matmul's psum-bank WAR engine equals its
  rhs-producer engine (m-banks: vector, l-banks: scalar), so with the
  two dummy matmuls that absorb DMA queue ticks, every matmul needs at
  most one new sync wait.
- Activations are [feature-in-block on partition, (block, batch) free];
  each layer's psum lands exactly in the next layer's rhs layout — no
  transposes or relayouts anywhere.  fp16 weights/activations (fp32
  PSUM): ~2.7e-3 rel err vs the 2e-2 gate.
"""

import numpy as np

import concourse.bacc as bacc
import concourse.bass as bass
import concourse.mybir as mybir
import concourse.tile as tile
from bass_rust import add_dep_helper
from concourse.bass_utils import run_bass_kernel_spmd

NCORES = 8
EPS = 1e-5
F32 = mybir.dt.float32
F16 = mybir.dt.float16
AF = mybir.ActivationFunctionType
ALU = mybir.AluOpType

# blob1 [128, *] fp16, split for early compute start:
#   blob1a: kron | xt | biasT | w3 clusters 0-31 (groups 0-3)
#   blob1b: w3 clusters 32-63 (groups 4-7)
C_KR = 0            # kron(I16, ones(1,32)) replicated at rows {0,32,64}
C_XT = 512          # x tile t: rows 32*(t%3), cols C_XT + 32*(t//3)   [43 stripes]
C_BT = 1888         # biasT m-bank g: rows 32*(g%3)+(0..16), cols C_BT+128*(g//3)
C_BTL = 2272        # biasT l-bank g (direct mode only): same layout
C_W3 = 2656         # band t, br: rows 32*(t%3), col block 2*(t//3)+br
N1A = 2656 + 22 * 2 * 128   # w3 blocks b3=0..21 (bands 0-65)
N1B = 21 * 2 * 128          # w3 blocks b3=22..42
# blob2 [128, N2] fp16: w2/w1/w0 k-chunk stationaries
C_W2 = 0            # (G, br, j, c): ((G*2+br)*8 + j*2 + c)*32    [16 G]
C_W1 = 8192         # (G2, br, j, c): same layout                 [2 G2]
C_W0 = 9216         # (br, c): (br*2+c)*32
N2 = 9344
# blob3 [128, N3] fp32: drain bias columns + BN/out params
C_B2M = 0           # bm2_eff per psum group [16]
C_B1M = 16          # bm1_eff [2]
C_B0 = 18           # col 0 = bm0_eff, col 1 = l drain bias (rows 0-31)
C_B2D = 20          # direct-mode (1-g)*bl2 [16]
C_B1D = 36          # [2]
C_B2T = 38          # direct-mode plain bl2 (l-chain drain) [16]
C_B1T = 54          # [2]
C_AUG = 56          # [33, 2] = [Wout.T ; bout]
C_BN = 58           # col 58 = gamma', col 59 = beta (rows 0-31)
N3 = 60


def _d3_banks():
    """9 psum banks, each holding only bands with one partition-row residue
    (mixing row bases within one PSUM bank hangs the device).  Ordered so
    the blob1a/1b column split is crossed as late as possible."""
    banks = []
    start = 0
    for q in range(3):
        for r in range(3):
            size = 16 if q < 2 else (11, 11, 10)[r]
            ts = [3 * (16 * q + j) + r for j in range(size)]
            banks.append((r, q, size, start, ts))
            start += size
    return banks


_D3_BANKS = _d3_banks()
_D3_POS = {t: start + j
           for (r, q, size, start, ts) in _D3_BANKS
           for j, t in enumerate(ts)}


def _extract_blocks(w, n, rows, cols):
    """Diagonal blocks of block-diag matrix w: out[i] = w[i*rows:(i+1)*rows, i*cols:(i+1)*cols]."""
    s0, s1 = w.strides
    return np.lib.stride_tricks.as_strided(
        w, (n, rows, cols), (rows * s0 + cols * s1, s0, s1)
    ).copy()


def _build_module(scaled: bool, g: float, debug: bool = False) -> bass.Bass:
    nc = bacc.Bacc(num_devices=NCORES)

    in1a_d = nc.dram_tensor("in1a", [128, N1A], F16, kind="ExternalInput")
    in1b_d = nc.dram_tensor("in1b", [128, N1B], F16, kind="ExternalInput")
    in3_d = nc.dram_tensor("in3", [128, N3], F32, kind="ExternalInput")
    in2_d = nc.dram_tensor("in2", [128, N2], F16, kind="ExternalInput")
    out_d = nc.dram_tensor("out", [32, 2], F32, kind="ExternalOutput")
    dbg_d = {}
    if debug:
        for nm, shp in [("d_u3", [128, 4096]), ("d_xl3", [128, 4096]),
                        ("d_u2", [128, 512]), ("d_xl2", [128, 512]),
                        ("d_u1", [128, 64]), ("d_xl1", [128, 64]),
                        ("d_u0", [32, 32])]:
            dbg_d[nm] = nc.dram_tensor(nm, shp, F32, kind="ExternalOutput")

    eps_c = EPS / (1.0 - g) ** 2 if scaled else EPS
    sub_combine = (not scaled) and g < 0.0

    with tile.TileContext(nc) as tc:
        with (
            tc.tile_pool(name="weights", bufs=1) as wp,
            tc.tile_pool(name="acts", bufs=1) as acp,
            tc.tile_pool(name="scratch", bufs=2) as sp,
            tc.tile_pool(name="small", bufs=2) as smp,
            tc.tile_pool(name="psM", bufs=2, space="PSUM") as pM,
            tc.tile_pool(name="psL", bufs=2, space="PSUM") as pL,
            tc.tile_pool(name="psSm", bufs=2, space="PSUM") as pSm,
            tc.tile_pool(name="psSl", bufs=2, space="PSUM") as pSl,
        ):
            in1a = wp.tile([128, N1A], F16, name="in1a_sb")
            dma1a = nc.sync.dma_start(in1a[:, :], in1a_d[:, :])
            in1b = wp.tile([128, N1B], F16, name="in1b_sb")
            dma1b = nc.sync.dma_start(in1b[:, :], in1b_d[:, :])
            in3 = wp.tile([128, N3], F32, name="in3_sb")
            dma3 = nc.sync.dma_start(in3[:, :], in3_d[:, :])
            in2 = wp.tile([128, N2], F16, name="in2_sb")
            dma2 = nc.sync.dma_start(in2[:, :], in2_d[:, :])
            add_dep_helper(dma1b.ins, dma1a.ins, False, "queue order 1a->1b")
            add_dep_helper(dma3.ins, dma1b.ins, False, "queue order 1b->3")
            add_dep_helper(dma2.ins, dma3.ins, False, "queue order 3->2")

            def xt_t(t):
                r, q = t % 3, t // 3
                return in1a[32 * r : 32 * r + 32, C_XT + 32 * q : C_XT + 32 * q + 32]

            def w3ap(t, br):
                r, b3 = t % 3, t // 3
                blk = 2 * b3 + br
                if b3 < 22:
                    return in1a[32 * r : 32 * r + 32,
                                C_W3 + 128 * blk : C_W3 + 128 * (blk + 1)]
                o = 128 * (blk - 44)
                return in1b[32 * r : 32 * r + 32, o : o + 128]

            def btap(r, q, size, base=C_BT):
                return in1a[32 * r : 32 * r + size,
                            base + 128 * q : base + 128 * q + 128]

            def krap(r, size):
                return in1a[32 * r : 32 * r + size, C_KR : C_KR + 32 * size]

            def w2ap(G, br, j, c):
                o = C_W2 + ((G * 2 + br) * 8 + j * 2 + c) * 32
                return in2[:, o : o + 32]

            def w1ap(G2, br, j, c):
                o = C_W1 + ((G2 * 2 + br) * 8 + j * 2 + c) * 32
                return in2[:, o : o + 32]

            def w0ap(br, c):
                o = C_W0 + (br * 2 + c) * 32
                return in2[:, o : o + 32]

            # persistent activations: [feature-in-block on partition, (block, batch) free]
            u3 = acp.tile([128, 4096], F16, name="u3_sb")
            xl3 = acp.tile([128, 4096], F16, name="xl3_sb")
            u2 = acp.tile([128, 512], F16, name="u2_sb")
            xl2 = acp.tile([128, 512], F16, name="xl2_sb")
            u1 = acp.tile([128, 64], F16, name="u1_sb")
            xl1 = acp.tile([128, 64], F16, name="xl1_sb")

            # warm the scalar engine's activation table early (one-time ~1.3us)
            eps_t = wp.tile([32, 1], F32, name="eps_t")
            nc.vector.memset(eps_t[:, :], eps_c)
            warm = smp.tile([32, 1], F32, name="warm", tag="warm")
            nc.scalar.activation(warm[:, :], eps_t[:, :], AF.Sqrt, bias=eps_t[:, :],
                                 scale=1.0)

            # ---- depth-3: 9 pure-residue banks (bias-MM + band MMs) ----
            for bk, (r, q, size, start, ts) in enumerate(_D3_BANKS):
                if bk == 3:
                    # absorb blob1b's queue tick onto PE before its first use
                    psd2 = pSm.tile([32, 2], F32, name="psd2", tag="m")
                    nc.tensor.matmul(
                        psd2[:, :], lhsT=in1b[0:32, 0:32], rhs=in1b[0:32, 0:2],
                        start=True, stop=True,
                    )
                ns = 32 * size
                bm = pM.tile([128, 512], F32, name="bm3", tag="psM")
                bl = pL.tile([128, 512], F32, name="bl3", tag="psL")
                nc.tensor.matmul(
                    bm[:, 0:ns], lhsT=btap(r, q, size), rhs=krap(r, size),
                    start=True, stop=False, tile_position=(32 * r, 0),
                    skip_group_check=True,
                )
                for j, t in enumerate(ts):
                    nc.tensor.matmul(
                        bm[:, 32 * j : 32 * (j + 1)], lhsT=w3ap(t, 0),
                        rhs=xt_t(t), start=False, stop=True,
                        tile_position=(32 * r, 0), skip_group_check=True,
                    )
                if not scaled:
                    # direct mode: psl must carry bl3 (true x_linear chain)
                    nc.tensor.matmul(
                        bl[:, 0:ns], lhsT=btap(r, q, size, C_BTL),
                        rhs=krap(r, size),
                        start=True, stop=False, tile_position=(32 * r, 0),
                        skip_group_check=True,
                    )
                for j, t in enumerate(ts):
                    nc.tensor.matmul(
                        bl[:, 32 * j : 32 * (j + 1)], lhsT=w3ap(t, 1),
                        rhs=xt_t(t), start=scaled, stop=True,
                        tile_position=(32 * r, 0),
                        skip_group_check=not scaled,
                    )
                sl = slice(32 * start, 32 * (start + size))
                xl3s = xl3[:, sl]
                # l: pure cast on scalar (bias deferred downstream)
                nc.scalar.activation(xl3s, bl[:, 0:ns], AF.Copy)
                hm = sp.tile([128, 512], F16, name="hm3", tag="hm3")
                nc.vector.tensor_scalar_max(hm[:, 0:ns], bm[:, 0:ns], 0.0)
                if scaled:
                    nc.vector.tensor_add(u3[:, sl], hm[:, 0:ns], xl3s)
                else:
                    xlc = sp.tile([128, 512], F16, name="xlc3", tag="xlc3")
                    nc.vector.tensor_scalar_mul(xlc[:, 0:ns], bl[:, 0:ns], 1.0 - g)
                    if sub_combine:
                        nc.vector.tensor_sub(u3[:, sl], xlc[:, 0:ns], hm[:, 0:ns])
                    else:
                        nc.vector.tensor_add(u3[:, sl], hm[:, 0:ns], xlc[:, 0:ns])

            # absorb blob3+blob2 queue ticks onto PE before depth-2
            psd1 = pSm.tile([32, 2], F32, name="psd1", tag="m")
            nc.tensor.matmul(
                psd1[:, :], lhsT=in2[0:32, 0:32], rhs=in2[0:32, 0:2],
                start=True, stop=True,
            )

            def mid_drains(psm, psl, bmcol, udst, xldst, blccol, bltcol, hmtag,
                           rows):
                """m: relu(z+bias) on vector; l: pure cast on scalar; combine on vector."""
                hm = smp.tile([rows, 32], F16, name=hmtag, tag=hmtag)
                nc.vector.tensor_scalar(
                    hm[:, :], psm, bmcol, 0.0, op0=ALU.add, op1=ALU.max
                )
                if scaled:
                    nc.scalar.activation(xldst, psl, AF.Copy)
                    nc.vector.tensor_add(udst, hm[:, :], xldst)
                else:
                    # direct mode: true l chain, bias via columns (vector)
                    nc.vector.tensor_scalar(xldst, psl, bltcol, None, op0=ALU.add)
                    xlc = smp.tile([rows, 32], F16, name=hmtag + "c", tag=hmtag + "c")
                    nc.vector.tensor_scalar(
                        xlc[:, :], psl, 1.0 - g, blccol, op0=ALU.mult, op1=ALU.add
                    )
                    if sub_combine:
                        nc.vector.tensor_sub(udst, xlc[:, :], hm[:, :])
                    else:
                        nc.vector.tensor_add(udst, hm[:, :], xlc[:, :])

            # ---- depth-2: 16 psum groups of 4 nodes ----
            for G in range(16):
                psm = pSm.tile([128, 32], F32, name="ps2m", tag="m")
                psl = pSl.tile([128, 32], F32, name="ps2l", tag="l")
                for j in range(4):
                    n = 4 * G + j
                    for c in range(2):
                        tau = _D3_POS[2 * n + c]
                        nc.tensor.matmul(
                            psm[32 * j : 32 * (j + 1), :], lhsT=w2ap(G, 0, j, c),
                            rhs=u3[:, 32 * tau : 32 * (tau + 1)],
                            start=(c == 0), stop=(c == 1),
                            tile_position=(0, 32 * j),
                        )
                for j in range(4):
                    n = 4 * G + j
                    for c in range(2):
                        tau = _D3_POS[2 * n + c]
                        nc.tensor.matmul(
                            psl[32 * j : 32 * (j + 1), :], lhsT=w2ap(G, 1, j, c),
                            rhs=xl3[:, 32 * tau : 32 * (tau + 1)],
                            start=(c == 0), stop=(c == 1),
                            tile_position=(0, 32 * j),
                        )
                mid_drains(psm[:, :], psl[:, :],
                           in3[:, C_B2M + G : C_B2M + G + 1],
                           u2[:, 32 * G : 32 * (G + 1)],
                           xl2[:, 32 * G : 32 * (G + 1)],
                           in3[:, C_B2D + G : C_B2D + G + 1],
                           in3[:, C_B2T + G : C_B2T + G + 1], "hm2", 128)

            # ---- depth-1: 2 psum groups of 4 nodes ----
            for G2 in range(2):
                psm = pSm.tile([128, 32], F32, name="ps1m", tag="m")
                psl = pSl.tile([128, 32], F32, name="ps1l", tag="l")
                for j in range(4):
                    n = 4 * G2 + j
                    for c in range(2):
                        sg = 2 * n + c
                        nc.tensor.matmul(
                            psm[32 * j : 32 * (j + 1), :], lhsT=w1ap(G2, 0, j, c),
                            rhs=u2[:, 32 * sg : 32 * (sg + 1)],
                            start=(c == 0), stop=(c == 1),
                            tile_position=(0, 32 * j),
                        )
                for j in range(4):
                    n = 4 * G2 + j
                    for c in range(2):
                        sg = 2 * n + c
                        nc.tensor.matmul(
                            psl[32 * j : 32 * (j + 1), :], lhsT=w1ap(G2, 1, j, c),
                            rhs=xl2[:, 32 * sg : 32 * (sg + 1)],
                            start=(c == 0), stop=(c == 1),
                            tile_position=(0, 32 * j),
                        )
                mid_drains(psm[:, :], psl[:, :],
                           in3[:, C_B1M + G2 : C_B1M + G2 + 1],
                           u1[:, 32 * G2 : 32 * (G2 + 1)],
                           xl1[:, 32 * G2 : 32 * (G2 + 1)],
                           in3[:, C_B1D + G2 : C_B1D + G2 + 1],
                           in3[:, C_B1T + G2 : C_B1T + G2 + 1], "hm1", 128)

            # ---- depth-0 (root): one node, fp32 out for batchnorm ----
            ps0m = pSm.tile([32, 32], F32, name="ps0m", tag="m")
            ps0l = pSl.tile([32, 32], F32, name="ps0l", tag="l")
            for c in range(2):
                nc.tensor.matmul(
                    ps0m[:, :], lhsT=w0ap(0, c), rhs=u1[:, 32 * c : 32 * (c + 1)],
                    start=(c == 0), stop=(c == 1),
                )
            for c in range(2):
                nc.tensor.matmul(
                    ps0l[:, :], lhsT=w0ap(1, c), rhs=xl1[:, 32 * c : 32 * (c + 1)],
                    start=(c == 0), stop=(c == 1),
                )
            hm0 = smp.tile([32, 32], F32, name="hm0", tag="hm0")
            xl0 = smp.tile([32, 32], F32, name="xl0", tag="xl0")
            u0 = acp.tile([32, 32], F32, name="u0")
            nc.vector.tensor_scalar(
                hm0[:, :], ps0m[:, :], in3[0:32, C_B0 : C_B0 + 1], 0.0,
                op0=ALU.add, op1=ALU.max,
            )
            if scaled:
                nc.vector.tensor_scalar(
                    xl0[:, :], ps0l[:, :], in3[0:32, C_B0 + 1 : C_B0 + 2], None,
                    op0=ALU.add,
                )
                nc.vector.tensor_add(u0[:, :], hm0[:, :], xl0[:, :])
            else:
                nc.vector.tensor_scalar(
                    xl0[:, :], ps0l[:, :], 1.0 - g,
                    in3[0:32, C_B0 + 1 : C_B0 + 2], op0=ALU.mult, op1=ALU.add,
                )
                if sub_combine:
                    nc.vector.tensor_sub(u0[:, :], xl0[:, :], hm0[:, :])
                else:
                    nc.vector.tensor_add(u0[:, :], hm0[:, :], xl0[:, :])

            # ---- batchnorm over the batch (free) axis ----
            stats = smp.tile([32, 6], F32, name="stats", tag="stats")
            mv = smp.tile([32, 2], F32, name="mv", tag="mv")
            nc.vector.bn_stats(stats[:, :], u0[:, :])
            nc.vector.bn_aggr(mv[:, :], stats[:, :])
            sq = smp.tile([32, 1], F32, name="sq", tag="sq")
            nc.scalar.activation(
                sq[:, :], mv[:, 1:2], AF.Sqrt, bias=eps_t[:, :], scale=1.0
            )
            rstd = smp.tile([32, 1], F32, name="rstd", tag="rstd")
            nc.vector.reciprocal(rstd[:, :], sq[:, :])

            aug = acp.tile([33, 32], F32, name="aug")
            nc.vector.memset(aug[32:33, :], 1.0)
            nc.vector.tensor_scalar(
                aug[0:32, :], u0[:, :], mv[:, 0:1], rstd[:, :],
                op0=ALU.subtract, op1=ALU.mult,
            )
            nc.vector.tensor_scalar(
                aug[0:32, :], aug[0:32, :], in3[0:32, C_BN : C_BN + 1],
                in3[0:32, C_BN + 1 : C_BN + 2], op0=ALU.mult, op1=ALU.add,
            )
            pso = pSm.tile([32, 2], F32, name="pso", tag="m")
            nc.tensor.matmul(
                pso[:, :], lhsT=aug[:, :], rhs=in3[0:33, C_AUG : C_AUG + 2],
                start=True, stop=True,
            )
            outt = smp.tile([32, 2], F32, name="outt", tag="outt")
            nc.vector.tensor_copy(outt[:, :], pso[:, :])
            nc.gpsimd.dma_start(out_d[:, :], outt[:, :])
            if debug:
                for src, nm in [(u3[:, :], "d_u3"), (xl3[:, :], "d_xl3"),
                                (u2[:, :], "d_u2"), (xl2[:, :], "d_xl2"),
                                (u1[:, :], "d_u1"), (xl1[:, :], "d_xl1"),
                                (u0[:, :], "d_u0")]:
                    nc.gpsimd.dma_start(dbg_d[nm][:, :], src)

    nc.finalize()
    return nc


_module_cache: dict = {}


def _get_module(scaled: bool, g: float, debug: bool = False) -> bass.Bass:
    key = (scaled, round(float(g), 12), debug)
    if key not in _module_cache:
        _module_cache[key] = _build_module(scaled, g, debug)
    return _module_cache[key]


def _pack_inputs(x, Wm3, bm3, Wl3, bl3, Wm2, bm2, Wl2, bl2, Wm1, bm1, Wl1, bl1,
                 Wm0, bm0, Wl0, bl0, gate, bn_gamma, bn_beta, Wout, bout,
                 scaled, g):
    f = np.float32
    if scaled:
        aW3 = g / (1.0 - g)  # relu-branch weight factor, d3 (raw input basis)
        aW = g               # relu-branch weight factor, d2/d1/d0 (u basis)
        ab = g / (1.0 - g)   # relu-branch bias factor, all layers
        sgn1mg = 1.0 if (1.0 - g) > 0 else -1.0
    else:
        aW3 = aW = ab = abs(g)
        sgn1mg = 1.0

    x = np.asarray(x, f)
    bm3 = np.asarray(bm3, f); bl3 = np.asarray(bl3, f)
    bm2 = np.asarray(bm2, f); bl2 = np.asarray(bl2, f)
    bm1 = np.asarray(bm1, f); bl1 = np.asarray(bl1, f)
    bm0 = np.asarray(bm0, f); bl0 = np.asarray(bl0, f)
    Wm0 = np.asarray(Wm0, f); Wl0 = np.asarray(Wl0, f)

    bl3m = _extract_blocks(np.asarray(Wm3, f), 128, 128, 32)  # (band, 128m, 32k)
    bl3l = _extract_blocks(np.asarray(Wl3, f), 128, 128, 32)
    bl2m = _extract_blocks(np.asarray(Wm2, f), 64, 32, 256)   # (n, 32m, 256k)
    bl2l = _extract_blocks(np.asarray(Wl2, f), 64, 32, 256)
    bl1m = _extract_blocks(np.asarray(Wm1, f), 8, 32, 256)
    bl1l = _extract_blocks(np.asarray(Wl1, f), 8, 32, 256)

    # l-branch bias accumulators (bias deferred through the bias-free l chain)
    bl2_acc = bl2 + np.einsum("nij,nj->ni", bl2l, bl3.reshape(64, 256)).reshape(-1)
    bl1_acc = bl1 + np.einsum("nij,nj->ni", bl1l, bl2_acc.reshape(8, 256)).reshape(-1)
    bl0_acc = bl0 + Wl0 @ bl1_acc
    if scaled:
        # m-branch drain biases absorb aW * Wm @ bl_acc of the previous level
        bm2_eff = ab * bm2 + aW * np.einsum(
            "nij,nj->ni", bl2m, bl3.reshape(64, 256)).reshape(-1)
        bm1_eff = ab * bm1 + aW * np.einsum(
            "nij,nj->ni", bl1m, bl2_acc.reshape(8, 256)).reshape(-1)
        bm0_eff = ab * bm0 + aW * (Wm0 @ bl1_acc)
        b0l_col = bl0_acc
    else:
        # direct mode stores true x / x_l (biases applied at each level)
        bm2_eff = ab * bm2
        bm1_eff = ab * bm1
        bm0_eff = ab * bm0
        b0l_col = (1.0 - g) * bl0

    # ---- blob1: kron | xt | biasT | w3 (4-high at bases 0/32/64/96) ----
    kron = np.kron(np.eye(16, dtype=f), np.ones((1, 32), f))   # [16, 512]
    tiles = x.T.reshape(128, 32, 32)                           # tile t = x[:,32t:+32].T
    tpad = np.zeros((129, 32, 32), f); tpad[:128] = tiles
    xt = tpad.reshape(43, 3, 32, 32).transpose(1, 2, 0, 3).reshape(96, 43 * 32)
    bT3 = (ab * bm3).reshape(128, 128)                         # [band, m]
    S3 = np.stack([aW3 * bl3m.transpose(0, 2, 1),
                   bl3l.transpose(0, 2, 1)], axis=1)           # (band, br, 32k, 128m)
    S3p = np.zeros((129, 2, 32, 128), f); S3p[:128] = S3
    # col block 2*b3+br holds bands 3*b3+e at rows 32e
    w3cols = (S3p.reshape(43, 3, 2, 32, 128)                   # (b3, e, br, k, m)
              .transpose(1, 3, 0, 2, 4)                        # (e, k, b3, br, m)
              .reshape(96, 43 * 2 * 128))

    blob1 = np.zeros((128, N1A + N1B), f)
    for r in range(3):
        blob1[32 * r : 32 * r + 16, C_KR : C_KR + 512] = kron
    blob1[0:96, C_XT : C_XT + 43 * 32] = xt
    bTl3 = bl3.reshape(128, 128)
    for (rr, q, size, start, ts) in _D3_BANKS:
        blob1[32 * rr : 32 * rr + size,
              C_BT + 128 * q : C_BT + 128 * q + 128] = bT3[ts]
        if not scaled:
            blob1[32 * rr : 32 * rr + size,
                  C_BTL + 128 * q : C_BTL + 128 * q + 128] = bTl3[ts]
    blob1[0:96, C_W3 :] = w3cols
    in1a = blob1[:, :N1A].astype(np.float16)
    in1b = np.ascontiguousarray(blob1[:, N1A:]).astype(np.float16)

    # ---- blob2: w2/w1/w0 [128, 32] k-chunk stationaries ----
    def mid_stationaries(bm_, bl_, nnodes):
        Sm = (aW * bm_).reshape(nnodes, 32, 2, 128).transpose(0, 2, 3, 1)
        Sl = bl_.reshape(nnodes, 32, 2, 128).transpose(0, 2, 3, 1)
        S = np.stack([Sm.reshape(nnodes // 4, 4, 2, 128, 32),
                      Sl.reshape(nnodes // 4, 4, 2, 128, 32)], axis=2)
        return (S.transpose(4, 0, 2, 1, 3, 5)                  # (k, G, br, j, c, m)
                .reshape(128, nnodes * 2 * 2 * 32))

    in2 = np.zeros((128, N2), f)
    in2[:, C_W2 : C_W2 + 8192] = mid_stationaries(bl2m, bl2l, 64)
    in2[:, C_W1 : C_W1 + 1024] = mid_stationaries(bl1m, bl1l, 8)
    S0 = np.stack([(aW * Wm0).reshape(32, 2, 128).transpose(1, 2, 0),
                   Wl0.reshape(32, 2, 128).transpose(1, 2, 0)], axis=0)
    in2[:, C_W0 : C_W0 + 128] = S0.transpose(2, 0, 1, 3).reshape(128, 128)
    in2 = in2.astype(np.float16)

    # ---- blob3: fp32 bias columns + BN/out ----
    in3 = np.zeros((128, N3), f)
    in3[:, C_B2M : C_B2M + 16] = bm2_eff.reshape(16, 128).T
    in3[:, C_B1M : C_B1M + 2] = bm1_eff.reshape(2, 128).T
    in3[0:32, C_B0] = bm0_eff
    in3[0:32, C_B0 + 1] = b0l_col
    in3[:, C_B2D : C_B2D + 16] = (1.0 - g) * bl2.reshape(16, 128).T
    in3[:, C_B1D : C_B1D + 2] = (1.0 - g) * bl1.reshape(2, 128).T
    in3[:, C_B2T : C_B2T + 16] = bl2.reshape(16, 128).T
    in3[:, C_B1T : C_B1T + 2] = bl1.reshape(2, 128).T
    in3[:32, C_AUG : C_AUG + 2] = np.asarray(Wout, f).T
    in3[32, C_AUG : C_AUG + 2] = np.asarray(bout, f)
    in3[:32, C_BN] = sgn1mg * np.asarray(bn_gamma, f)
    in3[:32, C_BN + 1] = np.asarray(bn_beta, f)

    im = {"in1a": in1a, "in1b": in1b, "in2": in2, "in3": in3}
    return [im for _ in range(NCORES)]


def kernel(x, Wm3, bm3, Wl3, bl3, Wm2, bm2, Wl2, bl2, Wm1, bm1, Wl1, bl1,
           Wm0, bm0, Wl0, bl0, gate, bn_gamma, bn_beta, Wout, bout,
           _trace=False, _trace_kwargs=None, _debug=False):
    g = float(np.asarray(gate))
    scaled = abs(1.0 - g) > 1e-6 and (g / (1.0 - g)) >= 0.0
    nc = _get_module(scaled, g, _debug)
    in_maps = _pack_inputs(
        x, Wm3, bm3, Wl3, bl3, Wm2, bm2, Wl2, bl2, Wm1, bm1, Wl1, bl1,
        Wm0, bm0, Wl0, bl0, gate, bn_gamma, bn_beta, Wout, bout, scaled, g,
    )
    kwargs = dict(_trace_kwargs or {})
    res = run_bass_kernel_spmd(
        nc, in_maps, core_ids=list(range(NCORES)), trace=_trace, **kwargs
    )
    out = np.asarray(res.results[0]["out"], np.float32)
    if _debug or _trace:
        return out, res
    return out
